# revision 28
# baseline (speedup 1.0000x reference)
"""BiDAF attention-flow kernel for Trainium2 (8 NeuronCores, data-parallel over batch).

Per core (one batch element):
  s[j,i]   = c[j] + q[i] + sum_h w_cq[h]*emb2[j,h]*emb1[i,h]
  a        = softmax_i(s)          (c[j] drops out of the row softmax)
  y2x      = a @ emb1
  b_att    = softmax_j(max_i s)
  x2y      = sum_j b_att[j]*emb2[j]
  out      = [emb2, y2x, emb2*y2x, emb2*x2y] @ w_red + b_red

Implementation notes:
  - b_c/b_q/b_cq cancel exactly in both softmaxes (row/column constants).
  - Row softmax uses a FIXED exp shift (s - SHIFT); true row max recovered as
    SHIFT + ln(max_i u) for b_att.
  - The s-matmul, y2x and pass-1 run on fp8(e4m3) with DoubleRow perf mode
    (2 K-planes per instruction, ~1.8x over bf16 at N=512).  q, c, pass-2 and
    u stay bf16: fp8 there costs ~1-3% output error for little speed.
    Weights are pre-scaled x16 on the host so their fp8 encodings stay in the
    normal range; the 1/16 descale is folded into the exp scale / output STTs.
  - Softmax normalization is folded into the PE transpose of u: transposing
    against diag(1/Z_j) instead of the identity yields normalized a^T free.
  - y2x is computed per GROUP of 4 j-tiles as N=512 DoubleRow sweeps (DR needs
    FD>=256 to beat fast-weight-load bf16).
  - Embeddings arrive TRANSPOSED from the host ([H, L]): DMA rows are then
    4KB (vs 1.5KB), ~2.7x fewer descriptors on the packet-rate-limited DMA
    queues, and e2^T needs no on-device transposing at all.  w1..w4 travel as
    one packed [H, 4*OUTP] tensor for the same reason.  Natural-layout copies
    (e1n for y2x, e2n for x2y) are re-derived by PE transposes, which are
    REGULAR matmuls against an identity: transpose-mode does not count as PE
    activity for the HAM clock gate, real matmuls do, so the load phase warms
    the clock to 2.4 GHz and in-loop e2n transposes keep it there.
  - Main loop is software-pipelined (A = s/exp/stats, T = u transposes, Y =
    grouped y2x, C = pass-1) so the in-order PE queue never head-blocks on a
    fresh dependency; idle >3.4us would re-throttle the clock to 1.2 GHz.
"""

import numpy as np
import ml_dtypes

P = 128
XL = 2048
YL = 2048
H = 768
OUT = 300
OUTP = 320      # OUT padded to a 16B-aligned fp8 stride for DoubleRow
NJT = YL // P   # 16 j tiles
NIC = XL // P   # 16 i chunks
NHC = H // P    # 6 h chunks
SLAB = 512
NSLAB = XL // SLAB  # 4
NCORES = 8
SHIFT = 2.0     # fixed exp shift; keeps u = exp(s-SHIFT) in fp8/bf16 range
WS = 16.0       # host-side weight scale (wq, wc, wcq, w1..w4)
BS = 64.0       # b_att fp8 scale

_CACHE = {}


def _fix_waits(nc, mybir, max_waits=1):
    """This walrus build rejects >1 sync wait per instruction.

    Pass 1: drop waits that are transitively implied by another wait on the
    same instruction.  Pass 2: hoist remaining extra waits onto same-engine
    NoOps inserted right before the instruction.
    """
    from collections import defaultdict

    blocks = [bb for f in nc.m.functions for bb in f.blocks]
    insts = [ins for bb in blocks for ins in bb.instructions]

    dma_types = ("InstDMACopy", "InstDmaTransposeAnt")
    eng_stream = defaultdict(list)
    queue_stream = defaultdict(list)
    sem_events = defaultdict(list)
    cum = defaultdict(int)
    for i, ins in enumerate(insts):
        eng_stream[str(ins.engine)].append(i)
        si = ins.sync_info
        if si and si.on_update:
            for u in si.on_update:
                cum[u.id] += u.update_value
                sem_events[u.id].append((cum[u.id], i))
                if type(ins).__name__ in dma_types:
                    queue_stream[u.id].append(i)

    def achiever(sem_id, val):
        for cv, i in sem_events.get(sem_id, []):
            if cv >= val:
                return i
        return None

    eng_pos, q_pos = {}, {}
    for e, lst in eng_stream.items():
        for k, i in enumerate(lst):
            eng_pos[i] = (e, k)
    for s, lst in queue_stream.items():
        for k, i in enumerate(lst):
            q_pos[i] = (s, k)

    memo = {}

    def implied(i):
        if i in memo:
            return memo[i]
        memo[i] = set()
        out = {i}
        ins = insts[i]
        if i in q_pos:
            s, k = q_pos[i]
            if k > 0:
                out |= implied(queue_stream[s][k - 1])
        e, k = eng_pos[i]
        j = k - 1
        while j >= 0:
            p = eng_stream[e][j]
            if type(insts[p]).__name__ in dma_types:
                j -= 1
                continue
            out |= implied(p)
            break
        si = ins.sync_info
        if si and si.on_wait:
            for w in si.on_wait:
                a = achiever(w.id, w.wait_value)
                if a is not None:
                    out |= implied(a)
        memo[i] = out
        return out

    # pass 1: redundancy elimination
    for i, ins in enumerate(insts):
        si = ins.sync_info
        if not (si and si.on_wait and len(si.on_wait) > max_waits):
            continue
        waits = list(si.on_wait)
        ach = [(w, achiever(w.id, w.wait_value)) for w in waits]
        keep = []
        for wi, (w, a) in enumerate(ach):
            red = False
            if a is not None:
                for wj, (w2, a2) in enumerate(ach):
                    if wi != wj and a2 is not None and a != a2 and a in implied(a2):
                        red = True
                        break
            if not red:
                keep.append(w)
        si.on_wait = keep

    # pass 2: hoist extras onto same-engine NoOps
    k = 0
    for bb in blocks:
        lst = bb.instructions
        i = 0
        while i < len(lst):
            ins = lst[i]
            si = ins.sync_info
            if si and si.on_wait and len(si.on_wait) > max_waits:
                waits = list(si.on_wait)
                extra, keep = waits[:-max_waits], waits[-max_waits:]
                si.on_wait = keep
                nops = []
                for w in extra:
                    nop = mybir.InstNoOp(name=f"I-waitfix-{k}", ins=[], outs=[])
                    k += 1
                    nop.engine = ins.engine
                    nop.sync_info = mybir.SyncInfo(on_wait=[w], on_update=[])
                    nops.append(nop)
                lst[i:i] = nops
                i += len(nops)
            i += 1


def _build():
    import concourse.bass as bass
    import concourse.tile as tile
    import concourse.mybir as mybir
    from concourse.masks import make_identity

    f32 = mybir.dt.float32
    bf16 = mybir.dt.bfloat16
    fp8 = mybir.dt.float8e4
    DR = mybir.MatmulPerfMode.DoubleRow
    MUL = mybir.AluOpType.mult
    ADD = mybir.AluOpType.add
    MAX = mybir.AluOpType.max
    EXP = mybir.ActivationFunctionType.Exp
    LN = mybir.ActivationFunctionType.Ln
    AXX = mybir.AxisListType.X

    nc = bass.Bass("TRN2", target_bir_lowering=False, debug=False,
                   num_devices=NCORES)

    emb1t_d = nc.dram_tensor("emb1t", [H, XL], bf16, kind="ExternalInput")
    emb2t_d = nc.dram_tensor("emb2t", [H, YL], bf16, kind="ExternalInput")
    wc_d = nc.dram_tensor("wc", [P, NHC], bf16, kind="ExternalInput")
    wq_d = nc.dram_tensor("wq", [P, NHC], bf16, kind="ExternalInput")
    wcq_d = nc.dram_tensor("wcq", [P, NHC], f32, kind="ExternalInput")
    wall_d = nc.dram_tensor("wall", [H, 4 * OUTP], bf16, kind="ExternalInput")
    bred_d = nc.dram_tensor("bred", [1, OUTP], f32, kind="ExternalInput")
    out_d = nc.dram_tensor("out", [YL, OUT], bf16, kind="ExternalOutput")

    with tile.TileContext(nc) as tc:
        with (
            tc.tile_pool(name="res", bufs=1) as res,        # resident data
            tc.tile_pool(name="stage", bufs=3) as stage,    # out staging
            tc.tile_pool(name="small", bufs=1) as small,    # stats etc
            tc.tile_pool(name="pss", bufs=2, space="PSUM") as pss,
            tc.tile_pool(name="ptp", bufs=2, space="PSUM") as ptp,
            tc.tile_pool(name="psy", bufs=2, space="PSUM") as psy,
            tc.tile_pool(name="pso", bufs=1, space="PSUM") as pso,
        ):
            # ---- constants ----
            ident16 = res.tile([P, P], bf16, tag="ident16")
            make_identity(nc, ident16)
            ident32 = res.tile([P, P], f32, tag="ident32")
            make_identity(nc, ident32)
            onesBS = res.tile([1, P], f32, tag="onesBS")
            nc.vector.memset(onesBS, BS)
            ones16 = res.tile([1, P], bf16, tag="ones16")
            nc.vector.memset(ones16, 1.0)
            identUS = res.tile([P, P], bf16, tag="identUS")
            nc.vector.tensor_scalar_mul(identUS, ident16, 128.0)
            negC = res.tile([P, 1], f32, tag="negC")
            nc.vector.memset(negC, -SHIFT)
            warm_sb = res.tile([P, OUTP], bf16, tag="warm_sb")
            nc.vector.memset(warm_sb, 0.0)

            # PE warm-up with REAL matmuls: HAM grants 2.4 GHz only after
            # ~3.4us of sustained activity; these bridge t=0 until the load
            # transposes / q matmuls take over.
            _warm_k = [0]

            def warm(n):
                for _ in range(n):
                    wk = _warm_k[0]
                    _warm_k[0] += 1
                    wps = pso.tile([P, OUTP], f32, tag="pso", name=f"warm{wk}")
                    nc.tensor.matmul(wps, ident16, warm_sb, start=True,
                                     stop=True, skip_group_check=True)

            warm(12)

            # ---- small weights ----
            wc_sb = res.tile([P, NHC], bf16, tag="wc")
            nc.sync.dma_start(out=wc_sb, in_=wc_d[:])
            wq_sb = res.tile([P, NHC], bf16, tag="wq")
            nc.sync.dma_start(out=wq_sb, in_=wq_d[:])
            wcq_sb = res.tile([P, NHC], f32, tag="wcq")
            nc.sync.dma_start(out=wcq_sb, in_=wcq_d[:])
            bred_bc = res.tile([P, OUTP], f32, tag="bred_bc")
            _bap = bred_d.ap()
            nc.sync.dma_start(out=bred_bc, in_=bass.AP(
                tensor=_bap.tensor, offset=_bap.offset,
                ap=[[0, P]] + list(_bap.ap[1:])))

            # ---- transposed embeddings: direct DMA, 4KB rows ----
            # e2tt bf16 resident; e2ts = e2tt * (16 w_cq) fp8; e1tt fp8.
            # e1n (fp8 natural pairs, y2x stationary) via PE transposes.
            # e2n (fp8 natural pairs, x2y stationary) transposed in-loop.
            e2tt = res.tile([P, NHC, YL], bf16, tag="e2tt")
            e2ts = res.tile([P, NHC, YL], fp8, tag="e2ts")
            e1tt = res.tile([P, NHC, XL], fp8, tag="e1tt")
            e1n = [res.tile([P, 2, H], fp8, tag=f"e1n{i}", name=f"e1n{i}")
                   for i in range(NIC // 2)]
            e2n = [res.tile([P, 2, H], fp8, tag=f"e2n{i}", name=f"e2n{i}")
                   for i in range(NJT // 2)]
            e1ttbq = res.tile([P, NHC, XL], bf16, tag="e1ttb")
            e1ttb = e1ttbq

            for hc in range(NHC):
                nc.sync.dma_start(out=e2tt[:, hc, :],
                                  in_=emb2t_d[hc * P:(hc + 1) * P, :])
                nc.vector.tensor_scalar_mul(
                    e2ts[:, hc, :], e2tt[:, hc, :], wcq_sb[:, hc:hc + 1])
            for hc in range(NHC):
                nc.sync.dma_start(out=e1ttb[:, hc, :],
                                  in_=emb1t_d[hc * P:(hc + 1) * P, :])

            # q row (bf16) and its partition-broadcast replica qbc:
            # the per-slab psum init "q" used to be a K=1 matmul per slab per
            # tile (0.9us/tile of PE); instead build 16q broadcast to all 128
            # partitions ONCE and add it on the vector engine per slab.
            qrow = small.tile([1, XL], bf16, tag="qrow")
            qbc = res.tile([P, XL], bf16, tag="qbc")
            for sl in range(NSLAB):
                ssl = slice(sl * SLAB, (sl + 1) * SLAB)
                qp = pss.tile([1, SLAB], f32, tag="pss", name=f"qp{sl}")
                for hc in range(NHC):
                    nc.tensor.matmul(qp, wq_sb[:, hc:hc + 1],
                                     e1ttb[:, hc, ssl],
                                     start=(hc == 0), stop=(hc == NHC - 1),
                                     skip_group_check=True)
                nc.any.tensor_copy(out=qrow[:, ssl], in_=qp)
                qbp = pss.tile([P, SLAB], f32, tag="pss", name=f"qbp{sl}")
                nc.tensor.matmul(qbp, ones16, qrow[:, ssl],
                                 start=True, stop=True, skip_group_check=True)
                nc.any.tensor_copy(out=qbc[:, ssl], in_=qbp)
                warm(1)

            # e1tt fp8 cast, split across engines
            cast_engs = [nc.vector, nc.gpsimd]
            for hc in range(NHC):
                for half in range(2):
                    hsl = slice(half * XL // 2, (half + 1) * XL // 2)
                    cast_engs[(2 * hc + half) % 2].tensor_copy(
                        out=e1tt[:, hc, hsl], in_=e1ttb[:, hc, hsl])

            # packed w1..w4 [H, 4*OUTP] bf16; w2/w3 also cast fp8 for DR
            w_all = res.tile([P, NHC, 4 * OUTP], bf16, tag="w_all")
            w2q = res.tile([P, NHC, OUTP], fp8, tag="w2q")
            w3q = res.tile([P, NHC, OUTP], fp8, tag="w3q")
            for hc in range(NHC):
                nc.sync.dma_start(out=w_all[:, hc, :],
                                  in_=wall_d[hc * P:(hc + 1) * P, :])
            for hc in range(NHC):
                nc.any.tensor_copy(out=w2q[:, hc, :],
                                   in_=w_all[:, hc, OUTP:2 * OUTP])
                nc.any.tensor_copy(out=w3q[:, hc, :],
                                   in_=w_all[:, hc, 2 * OUTP:3 * OUTP])

            # ---- stats tiles ----
            M_sb = small.tile([P, NJT], f32, tag="M")
            c_sb = small.tile([P, NJT], f32, tag="c_sb")
            Z_sb = small.tile([P, NJT], f32, tag="Z")
            rZ_sb = small.tile([P, NJT], f32, tag="rZ")
            out_sb = res.tile([P, NJT, OUTP], f32, tag="out_sb")

            # all c columns up-front (PE filler during the load):
            # c^T = e2tt^T @ (16 w_c), then /16
            for jt in range(NJT):
                jsl = slice(jt * P, (jt + 1) * P)
                cp = pss.tile([P, 1], f32, tag="pss", name=f"cp{jt}")
                for hc in range(NHC):
                    nc.tensor.matmul(cp, e2tt[:, hc, jsl], wc_sb[:, hc:hc + 1],
                                     start=(hc == 0), stop=(hc == NHC - 1),
                                     skip_group_check=True)
                nc.any.tensor_scalar_mul(c_sb[:, jt:jt + 1], cp, 1.0 / WS)

            # natural-layout transposes, drip-fed into the loop as PE filler:
            # all of e1n (y2x stationary) must land before Y(0); e2n (x2y
            # stationary) is only needed in the tail.
            trans_q = ([("e1", ic) for ic in range(NIC)] +
                       [("e2", jc) for jc in range(NJT)])

            def drip_trans(n):
                for _ in range(n):
                    if not trans_q:
                        return
                    kind, ck = trans_q.pop(0)
                    src_tt = e1ttbq if kind == "e1" else e2tt
                    dst = e1n if kind == "e1" else e2n
                    for b in range(2):
                        ps = ptp.tile([P, 3, P], f32, tag="ptp",
                                      name=f"nt{kind}{ck}_{b}")
                        for k in range(3):
                            hc = 3 * b + k
                            nc.tensor.matmul(
                                ps[:, k, :],
                                src_tt[:, hc, ck * P:(ck + 1) * P],
                                ident16, start=True, stop=True,
                                skip_group_check=True)
                        nc.any.tensor_copy(
                            out=dst[ck // 2][:, ck % 2,
                                             3 * b * P:(3 * b + 3) * P],
                            in_=ps)

            # ---- main loop: software-pipelined, y2x grouped by 4 tiles ----
            sjt_cm = tc.tile_pool(name="sjt", bufs=2)
            sjt = sjt_cm.__enter__()
            sg4_cm = tc.tile_pool(name="sg4", bufs=2)
            sg4 = sg4_cm.__enter__()
            tiles = {}
            gtiles = {}

            def a_phase(jt):
                jsl = slice(jt * P, (jt + 1) * P)
                # 16*s = 16*q + (e2*16wcq) @ e1^T; u = exp(16s/16 - SHIFT)
                u = sjt.tile([P, XL], bf16, tag="u", name=f"u{jt}")
                Zp = sjt.tile([P, NSLAB], f32, tag="Zp", name=f"Zp{jt}")
                for sl in range(NSLAB):
                    ssl = slice(sl * SLAB, (sl + 1) * SLAB)
                    sp = pss.tile([P, SLAB], f32, tag="pss", name=f"sp{jt}_{sl}")
                    for hp in range(NHC // 2):
                        nc.tensor.matmul(
                            sp, e2ts[:, 2 * hp:2 * hp + 2, jsl],
                            e1tt[:, 2 * hp:2 * hp + 2, ssl],
                            start=(hp == 0), stop=(hp == NHC // 2 - 1),
                            perf_mode=DR, skip_group_check=True)
                    # q added on the vector engine (broadcast replica)
                    nc.vector.tensor_tensor(out=sp, in0=sp,
                                            in1=qbc[:, ssl], op=ADD)
                    nc.scalar.activation(out=u[:, ssl], in_=sp, func=EXP,
                                         bias=negC, scale=1.0 / WS,
                                         accum_out=Zp[:, sl:sl + 1])
                drip_trans(4 if jt < 5 else 2)
                nc.vector.tensor_reduce(out=Z_sb[:, jt:jt + 1], in_=Zp,
                                        axis=AXX, op=ADD)
                nc.vector.reciprocal(out=rZ_sb[:, jt:jt + 1],
                                     in_=Z_sb[:, jt:jt + 1])

                # row max for b_att: M = c + SHIFT + ln(max u)
                umax = sjt.tile([P, 1], f32, tag="umax", name=f"umax{jt}")
                nc.vector.tensor_reduce(out=umax, in_=u, axis=AXX, op=MAX)
                lnu = sjt.tile([P, 1], f32, tag="lnu", name=f"lnu{jt}")
                nc.scalar.activation(out=lnu, in_=umax, func=LN)
                nc.vector.scalar_tensor_tensor(
                    out=M_sb[:, jt:jt + 1], in0=lnu, scalar=SHIFT,
                    in1=c_sb[:, jt:jt + 1], op0=ADD, op1=ADD)

                # normalized a^T transpose operand: diag(128/Z_j) -- the
                # x128 keeps small attention weights out of fp8 denormals
                diag = sjt.tile([P, P], bf16, tag="diag", name=f"diag{jt}")
                nc.vector.tensor_scalar_mul(diag, identUS, rZ_sb[:, jt:jt + 1])
                tiles[jt] = (u, diag)

            def get_uT4(g):
                if g not in gtiles:
                    gtiles[g] = sg4.tile([P, NIC, 4, P], fp8, tag="uT4",
                                         name=f"uT4_{g}")
                return gtiles[g]

            def t_phase(jt):
                u, diag = tiles.pop(jt)
                uT4 = get_uT4(jt // 4)
                jj = jt % 4
                # uT4[i, ic, jj, j] = u[j, i] * rZ_j, fp8 cast on the copy
                for g in range(NIC // 4):
                    tp = ptp.tile([P, 4, P], f32, tag="ptp", name=f"tp{jt}_{g}")
                    for k in range(4):
                        ic = g * 4 + k
                        nc.tensor.matmul(tp[:, k, :], u[:, ic * P:(ic + 1) * P],
                                         diag, start=True, stop=True,
                                         skip_group_check=True)
                    nc.any.tensor_copy(out=uT4[:, g * 4:(g + 1) * 4, jj, :],
                                       in_=tp)

            def y_phase(g):
                uT4 = gtiles.pop(g)
                gsl = slice(g * 4 * P, (g + 1) * 4 * P)
                y2xT4 = sg4.tile([P, NHC, 4 * P], fp8, tag="y2xT4",
                                 name=f"y2xT4_{g}")
                bl34 = sg4.tile([P, NHC, 4 * P], fp8, tag="bl34",
                                name=f"bl34_{g}")
                for hc in range(NHC):
                    yp = psy.tile([P, 4 * P], f32, tag="psy", name=f"yp{g}_{hc}")
                    for icp in range(NIC // 2):
                        nc.tensor.matmul(
                            yp,
                            e1n[icp][:, :, hc * P:(hc + 1) * P],
                            uT4[:, 2 * icp:2 * icp + 2, :, :],
                            start=(icp == 0), stop=(icp == NIC // 2 - 1),
                            perf_mode=DR, skip_group_check=True)
                    nc.vector.tensor_scalar_mul(y2xT4[:, hc, :], yp,
                                                 1.0 / 128.0)
                    nc.vector.tensor_mul(bl34[:, hc, :], e2tt[:, hc, gsl],
                                         y2xT4[:, hc, :])
                gtiles[(g, "y")] = (y2xT4, bl34)

            def c_phase(jt):
                g, jj = jt // 4, jt % 4
                y2xT4, bl34 = gtiles[(g, "y")]
                jsl4 = slice(jj * P, (jj + 1) * P)
                # pass-1 reduction: [y2x; e2*y2x] @ 16*[w2; w3] (DoubleRow)
                op1 = pso.tile([P, OUTP], f32, tag="pso", name=f"op1_{jt}")
                for hp in range(NHC // 2):
                    nc.tensor.matmul(op1, y2xT4[:, 2 * hp:2 * hp + 2, jsl4],
                                     w2q[:, 2 * hp:2 * hp + 2, :],
                                     start=(hp == 0), stop=False,
                                     perf_mode=DR, skip_group_check=True)
                for hp in range(NHC // 2):
                    nc.tensor.matmul(op1, bl34[:, 2 * hp:2 * hp + 2, jsl4],
                                     w3q[:, 2 * hp:2 * hp + 2, :],
                                     start=False, stop=(hp == NHC // 2 - 1),
                                     perf_mode=DR, skip_group_check=True)
                # out_sb = psum/16 + b_red
                nc.vector.scalar_tensor_tensor(
                    out=out_sb[:, jt, :], in0=op1, scalar=1.0 / WS,
                    in1=bred_bc, op0=MUL, op1=ADD)
                if jj == 3:
                    gtiles.pop((g, "y"))

            # prologue: A0..A4 interleaved with T0..T3
            a_phase(0)
            for jt in range(1, 5):
                a_phase(jt)
                t_phase(jt - 1)
            for g in range(4):
                y_phase(g)
                if g < 3:
                    for jj in range(4):
                        jt = 4 * (g + 1) + jj
                        if jt + 1 < NJT:
                            a_phase(jt + 1)
                        t_phase(jt)
                for jj in range(4):
                    c_phase(4 * g + jj)

            sg4_cm.__exit__(None, None, None)
            sjt_cm.__exit__(None, None, None)
            # bridge the serial b_att chain so the HAM clock stays warm
            warm(6)
            post_cm = tc.tile_pool(name="post", bufs=1)
            post = post_cm.__enter__()

            # ---- b_att = softmax_j(M), no max shift (M <= ~14, f32 exp ok) ----
            bexp = post.tile([P, NJT], f32, tag="bexp")
            brow = post.tile([P, 1], f32, tag="brow")
            nc.scalar.activation(out=bexp, in_=M_sb, func=EXP, accum_out=brow)
            tpb = pss.tile([1, P], f32, tag="pss", name="tpb")
            nc.tensor.transpose(tpb, brow, ident32)
            brw = post.tile([1, P], f32, tag="brw")
            nc.vector.tensor_copy(out=brw, in_=tpb)
            bs0 = post.tile([1, 1], f32, tag="bs0")
            nc.vector.tensor_reduce(out=bs0, in_=brw, axis=AXX, op=ADD)
            rb0 = post.tile([1, 1], f32, tag="rb0")
            nc.vector.reciprocal(rb0, bs0)
            # broadcast 64/sum(b) to all partitions with a K=1 matmul
            rbp = pss.tile([P, 1], f32, tag="pss", name="rbp")
            nc.tensor.matmul(rbp, onesBS, rb0, start=True, stop=True,
                             skip_group_check=True)
            rbz = post.tile([P, 1], f32, tag="rbz")
            nc.vector.tensor_copy(out=rbz, in_=rbp)
            battq = post.tile([P, NJT], fp8, tag="battq")
            nc.vector.tensor_scalar_mul(battq, bexp, rbz)

            # x2y^T directly: x2yT[h] = sum_j e2n[j,h] * (64 b_j), then /64
            x2p = psy.tile([P, NHC], f32, tag="psy", name="x2p")
            for hc in range(NHC):
                for jc in range(NJT):
                    nc.tensor.matmul(
                        x2p[:, hc:hc + 1],
                        e2n[jc // 2][:, jc % 2, hc * P:(hc + 1) * P],
                        battq[:, jc:jc + 1],
                        start=(jc == 0), stop=(jc == NJT - 1),
                        skip_group_check=True)
                warm(1)
            x2yT = post.tile([P, NHC], f32, tag="x2yT")
            nc.vector.tensor_scalar_mul(x2yT, x2p, 1.0 / BS)

            # w14 = 16*(w1 + x2y*w4), bf16 (fp8 here costs ~3% output error)
            w14 = res.tile([P, NHC, OUTP], bf16, tag="w14")
            for hc in range(NHC):
                nc.vector.scalar_tensor_tensor(
                    out=w14[:, hc, :], in0=w_all[:, hc, 3 * OUTP:4 * OUTP],
                    scalar=x2yT[:, hc:hc + 1], in1=w_all[:, hc, 0:OUTP],
                    op0=MUL, op1=ADD)
            warm(4)

            # ---- pass 2: out += emb2 @ w14/16 (bf16), stream out ----
            for jt in range(NJT):
                jsl = slice(jt * P, (jt + 1) * P)
                op2 = pso.tile([P, OUTP], f32, tag="pso", name=f"op2_{jt}")
                for hc in range(NHC):
                    nc.tensor.matmul(op2, e2tt[:, hc, jsl], w14[:, hc, :],
                                     start=(hc == 0), stop=(hc == NHC - 1),
                                     skip_group_check=True)
                fin = stage.tile([P, OUTP], bf16, tag="fin", name=f"fin{jt}")
                nc.vector.scalar_tensor_tensor(
                    out=fin, in0=op2, scalar=1.0 / WS,
                    in1=out_sb[:, jt, :], op0=MUL, op1=ADD)
                nc.sync.dma_start(out=out_d[jsl, :], in_=fin[:, 0:OUT])
            post_cm.__exit__(None, None, None)

    return nc


def _get_nc(drain_fix=True):
    if "nc" not in _CACHE:
        _CACHE["nc"] = _build()
    if drain_fix and not _CACHE.get("drain_fixed"):
        import concourse.mybir as mybir
        _fix_waits(_CACHE["nc"], mybir, max_waits=1)
        _CACHE["drain_fixed"] = True
    return _CACHE["nc"]


def _prep_weights(w_c, w_q, w_cq, w_red, b_red):
    bf = ml_dtypes.bfloat16
    w_red = np.asarray(w_red, dtype=np.float32)

    wall = np.zeros((H, 4 * OUTP), np.float32)
    for k in range(4):
        wall[:, k * OUTP:k * OUTP + OUT] = w_red[k * H:(k + 1) * H]
    bredp = np.zeros((1, OUTP), np.float32)
    bredp[0, :OUT] = np.asarray(b_red, np.float32)
    return {
        "wc": np.ascontiguousarray(
            (np.asarray(w_c, np.float32) * WS).reshape(NHC, P).T.astype(bf)),
        "wq": np.ascontiguousarray(
            (np.asarray(w_q, np.float32) * WS).reshape(NHC, P).T.astype(bf)),
        "wcq": np.ascontiguousarray(
            (np.asarray(w_cq, np.float32) * WS).reshape(NHC, P).T),
        "wall": np.ascontiguousarray((wall * WS).astype(bf)),
        "bred": np.ascontiguousarray(bredp),
    }


def kernel(emb1, emb2, w_c, b_c, w_q, b_q, w_cq, b_cq, w_red, b_red):
    from concourse.bass_utils import run_bass_kernel_spmd

    nc = _get_nc()
    bf = ml_dtypes.bfloat16

    # host-side transpose: DMA rows become 4KB (packet-rate-limited queues)
    emb1t = np.ascontiguousarray(
        np.asarray(emb1, dtype=np.float32).transpose(0, 2, 1).astype(bf))
    emb2t = np.ascontiguousarray(
        np.asarray(emb2, dtype=np.float32).transpose(0, 2, 1).astype(bf))

    # b_c, b_q, b_cq cancel exactly in both softmaxes (per-row/col consts).
    prep = _prep_weights(w_c, w_q, w_cq, w_red, b_red)

    in_maps = []
    for b in range(NCORES):
        in_maps.append({"emb1t": emb1t[b], "emb2t": emb2t[b], **prep})
    res = run_bass_kernel_spmd(nc, in_maps, core_ids=list(range(NCORES)))
    return np.stack([res.results[i]["out"] for i in range(NCORES)],
                    axis=0).astype(np.float32)


# revision 29
# speedup vs baseline: 1.0139x; 1.0139x over previous
"""BiDAF attention-flow kernel for Trainium2 (8 NeuronCores, data-parallel over batch).

Per core (one batch element):
  s[j,i]   = c[j] + q[i] + sum_h w_cq[h]*emb2[j,h]*emb1[i,h]
  a        = softmax_i(s)          (c[j] drops out of the row softmax)
  y2x      = a @ emb1
  b_att    = softmax_j(max_i s)
  x2y      = sum_j b_att[j]*emb2[j]
  out      = [emb2, y2x, emb2*y2x, emb2*x2y] @ w_red + b_red

Implementation notes:
  - b_c/b_q/b_cq cancel exactly in both softmaxes (row/column constants).
  - Row softmax uses a FIXED exp shift (s - SHIFT); true row max recovered as
    SHIFT + ln(max_i u) for b_att.
  - The s-matmul, y2x and pass-1 run on fp8(e4m3) with DoubleRow perf mode
    (2 K-planes per instruction, ~1.8x over bf16 at N=512).  q, c, pass-2 and
    u stay bf16: fp8 there costs ~1-3% output error for little speed.
    Weights are pre-scaled x16 on the host so their fp8 encodings stay in the
    normal range; the 1/16 descale is folded into the exp scale / output STTs.
  - Softmax normalization is folded into the PE transpose of u: transposing
    against diag(1/Z_j) instead of the identity yields normalized a^T free.
  - y2x is computed per GROUP of 4 j-tiles as N=512 DoubleRow sweeps (DR needs
    FD>=256 to beat fast-weight-load bf16).
  - Embeddings arrive TRANSPOSED from the host ([H, L]): DMA rows are then
    4KB (vs 1.5KB), ~2.7x fewer descriptors on the packet-rate-limited DMA
    queues, and e2^T needs no on-device transposing at all.  w1..w4 travel as
    one packed [H, 4*OUTP] tensor for the same reason.  Natural-layout copies
    (e1n for y2x, e2n for x2y) are re-derived by PE transposes, which are
    REGULAR matmuls against an identity: transpose-mode does not count as PE
    activity for the HAM clock gate, real matmuls do, so the load phase warms
    the clock to 2.4 GHz and in-loop e2n transposes keep it there.
  - Main loop is software-pipelined (A = s/exp/stats, T = u transposes, Y =
    grouped y2x, C = pass-1) so the in-order PE queue never head-blocks on a
    fresh dependency; idle >3.4us would re-throttle the clock to 1.2 GHz.
"""

import numpy as np
import ml_dtypes

P = 128
XL = 2048
YL = 2048
H = 768
OUT = 300
OUTP = 320      # OUT padded to a 16B-aligned fp8 stride for DoubleRow
NJT = YL // P   # 16 j tiles
NIC = XL // P   # 16 i chunks
NHC = H // P    # 6 h chunks
SLAB = 512
NSLAB = XL // SLAB  # 4
NCORES = 8
SHIFT = 2.0     # fixed exp shift; keeps u = exp(s-SHIFT) in fp8/bf16 range
WS = 16.0       # host-side weight scale (wq, wc, wcq, w1..w4)
BS = 64.0       # b_att fp8 scale

_CACHE = {}


def _fix_waits(nc, mybir, max_waits=1):
    """This walrus build rejects >1 sync wait per instruction.

    Pass 1: drop waits that are transitively implied by another wait on the
    same instruction.  Pass 2: hoist remaining extra waits onto same-engine
    NoOps inserted right before the instruction.
    """
    from collections import defaultdict

    blocks = [bb for f in nc.m.functions for bb in f.blocks]
    insts = [ins for bb in blocks for ins in bb.instructions]

    dma_types = ("InstDMACopy", "InstDmaTransposeAnt")
    eng_stream = defaultdict(list)
    queue_stream = defaultdict(list)
    sem_events = defaultdict(list)
    cum = defaultdict(int)
    for i, ins in enumerate(insts):
        eng_stream[str(ins.engine)].append(i)
        si = ins.sync_info
        if si and si.on_update:
            for u in si.on_update:
                cum[u.id] += u.update_value
                sem_events[u.id].append((cum[u.id], i))
                if type(ins).__name__ in dma_types:
                    queue_stream[u.id].append(i)

    def achiever(sem_id, val):
        for cv, i in sem_events.get(sem_id, []):
            if cv >= val:
                return i
        return None

    eng_pos, q_pos = {}, {}
    for e, lst in eng_stream.items():
        for k, i in enumerate(lst):
            eng_pos[i] = (e, k)
    for s, lst in queue_stream.items():
        for k, i in enumerate(lst):
            q_pos[i] = (s, k)

    memo = {}

    def implied(i):
        if i in memo:
            return memo[i]
        memo[i] = set()
        out = {i}
        ins = insts[i]
        if i in q_pos:
            s, k = q_pos[i]
            if k > 0:
                out |= implied(queue_stream[s][k - 1])
        e, k = eng_pos[i]
        j = k - 1
        while j >= 0:
            p = eng_stream[e][j]
            if type(insts[p]).__name__ in dma_types:
                j -= 1
                continue
            out |= implied(p)
            break
        si = ins.sync_info
        if si and si.on_wait:
            for w in si.on_wait:
                a = achiever(w.id, w.wait_value)
                if a is not None:
                    out |= implied(a)
        memo[i] = out
        return out

    # pass 1: redundancy elimination
    for i, ins in enumerate(insts):
        si = ins.sync_info
        if not (si and si.on_wait and len(si.on_wait) > max_waits):
            continue
        waits = list(si.on_wait)
        ach = [(w, achiever(w.id, w.wait_value)) for w in waits]
        keep = []
        for wi, (w, a) in enumerate(ach):
            red = False
            if a is not None:
                for wj, (w2, a2) in enumerate(ach):
                    if wi != wj and a2 is not None and a != a2 and a in implied(a2):
                        red = True
                        break
            if not red:
                keep.append(w)
        si.on_wait = keep

    # pass 2: hoist extras onto same-engine NoOps
    k = 0
    for bb in blocks:
        lst = bb.instructions
        i = 0
        while i < len(lst):
            ins = lst[i]
            si = ins.sync_info
            if si and si.on_wait and len(si.on_wait) > max_waits:
                waits = list(si.on_wait)
                extra, keep = waits[:-max_waits], waits[-max_waits:]
                si.on_wait = keep
                nops = []
                for w in extra:
                    nop = mybir.InstNoOp(name=f"I-waitfix-{k}", ins=[], outs=[])
                    k += 1
                    nop.engine = ins.engine
                    nop.sync_info = mybir.SyncInfo(on_wait=[w], on_update=[])
                    nops.append(nop)
                lst[i:i] = nops
                i += len(nops)
            i += 1


def _build():
    import concourse.bass as bass
    import concourse.tile as tile
    import concourse.mybir as mybir
    from concourse.masks import make_identity

    f32 = mybir.dt.float32
    bf16 = mybir.dt.bfloat16
    fp8 = mybir.dt.float8e4
    DR = mybir.MatmulPerfMode.DoubleRow
    MUL = mybir.AluOpType.mult
    ADD = mybir.AluOpType.add
    MAX = mybir.AluOpType.max
    EXP = mybir.ActivationFunctionType.Exp
    LN = mybir.ActivationFunctionType.Ln
    AXX = mybir.AxisListType.X

    nc = bass.Bass("TRN2", target_bir_lowering=False, debug=False,
                   num_devices=NCORES)

    emb1t_d = nc.dram_tensor("emb1t", [H, XL], bf16, kind="ExternalInput")
    emb2t_d = nc.dram_tensor("emb2t", [H, YL], bf16, kind="ExternalInput")
    wc_d = nc.dram_tensor("wc", [P, NHC], bf16, kind="ExternalInput")
    wq_d = nc.dram_tensor("wq", [P, NHC], bf16, kind="ExternalInput")
    wcq_d = nc.dram_tensor("wcq", [P, NHC], f32, kind="ExternalInput")
    wall_d = nc.dram_tensor("wall", [H, 4 * OUTP], bf16, kind="ExternalInput")
    bred_d = nc.dram_tensor("bred", [1, OUTP], f32, kind="ExternalInput")
    out_d = nc.dram_tensor("out", [YL, OUT], bf16, kind="ExternalOutput")

    with tile.TileContext(nc) as tc:
        with (
            tc.tile_pool(name="res", bufs=1) as res,        # resident data
            tc.tile_pool(name="stage", bufs=3) as stage,    # out staging
            tc.tile_pool(name="small", bufs=1) as small,    # stats etc
            tc.tile_pool(name="pss", bufs=2, space="PSUM") as pss,
            tc.tile_pool(name="ptp", bufs=2, space="PSUM") as ptp,
            tc.tile_pool(name="psy", bufs=2, space="PSUM") as psy,
            tc.tile_pool(name="pso", bufs=1, space="PSUM") as pso,
        ):
            # ---- constants ----
            ident16 = res.tile([P, P], bf16, tag="ident16")
            make_identity(nc, ident16)
            ident32 = res.tile([P, P], f32, tag="ident32")
            make_identity(nc, ident32)
            onesBS = res.tile([1, P], f32, tag="onesBS")
            nc.vector.memset(onesBS, BS)
            ones16 = res.tile([1, P], bf16, tag="ones16")
            nc.vector.memset(ones16, 1.0)
            identUS = res.tile([P, P], bf16, tag="identUS")
            nc.vector.tensor_scalar_mul(identUS, ident16, 128.0)
            negC = res.tile([P, 1], f32, tag="negC")
            nc.vector.memset(negC, -SHIFT)
            warm_sb = res.tile([P, OUTP], bf16, tag="warm_sb")
            nc.vector.memset(warm_sb, 0.0)

            # PE warm-up with REAL matmuls: HAM grants 2.4 GHz only after
            # ~3.4us of sustained activity; these bridge t=0 until the load
            # transposes / q matmuls take over.
            _warm_k = [0]

            def warm(n):
                for _ in range(n):
                    wk = _warm_k[0]
                    _warm_k[0] += 1
                    wps = pso.tile([P, OUTP], f32, tag="pso", name=f"warm{wk}")
                    nc.tensor.matmul(wps, ident16, warm_sb, start=True,
                                     stop=True, skip_group_check=True)

            warm(12)

            # ---- small weights ----
            wc_sb = res.tile([P, NHC], bf16, tag="wc")
            nc.sync.dma_start(out=wc_sb, in_=wc_d[:])
            wq_sb = res.tile([P, NHC], bf16, tag="wq")
            nc.sync.dma_start(out=wq_sb, in_=wq_d[:])
            wcq_sb = res.tile([P, NHC], f32, tag="wcq")
            nc.sync.dma_start(out=wcq_sb, in_=wcq_d[:])
            bred_bc = res.tile([P, OUTP], f32, tag="bred_bc")
            _bap = bred_d.ap()
            nc.sync.dma_start(out=bred_bc, in_=bass.AP(
                tensor=_bap.tensor, offset=_bap.offset,
                ap=[[0, P]] + list(_bap.ap[1:])))

            # ---- transposed embeddings: direct DMA, 4KB rows ----
            # e2tt bf16 resident; e2ts = e2tt * (16 w_cq) fp8; e1tt fp8.
            # e1n (fp8 natural pairs, y2x stationary) via PE transposes.
            # e2n (fp8 natural pairs, x2y stationary) transposed in-loop.
            e2tt = res.tile([P, NHC, YL], bf16, tag="e2tt")
            e2ts = res.tile([P, NHC, YL], fp8, tag="e2ts")
            e1tt = res.tile([P, NHC, XL], fp8, tag="e1tt")
            e1n = [res.tile([P, 2, H], fp8, tag=f"e1n{i}", name=f"e1n{i}")
                   for i in range(NIC // 2)]
            e2n = [res.tile([P, 2, H], fp8, tag=f"e2n{i}", name=f"e2n{i}")
                   for i in range(NJT // 2)]
            e1ttbq = res.tile([P, NHC, XL], bf16, tag="e1ttb")
            e1ttb = e1ttbq

            for hc in range(NHC):
                nc.sync.dma_start(out=e2tt[:, hc, :],
                                  in_=emb2t_d[hc * P:(hc + 1) * P, :])
                nc.vector.tensor_scalar_mul(
                    e2ts[:, hc, 0:YL // 2], e2tt[:, hc, 0:YL // 2],
                    wcq_sb[:, hc:hc + 1])
                nc.gpsimd.tensor_scalar(
                    e2ts[:, hc, YL // 2:YL], e2tt[:, hc, YL // 2:YL],
                    wcq_sb[:, hc:hc + 1], None, MUL)
            for hc in range(NHC):
                nc.sync.dma_start(out=e1ttb[:, hc, :],
                                  in_=emb1t_d[hc * P:(hc + 1) * P, :])

            # q row (bf16): q^T(x16) = (16 w_q)^T @ emb1^T
            qrow = small.tile([1, XL], bf16, tag="qrow")
            for sl in range(NSLAB):
                ssl = slice(sl * SLAB, (sl + 1) * SLAB)
                qp = pss.tile([1, SLAB], f32, tag="pss", name=f"qp{sl}")
                for hc in range(NHC):
                    nc.tensor.matmul(qp, wq_sb[:, hc:hc + 1],
                                     e1ttb[:, hc, ssl],
                                     start=(hc == 0), stop=(hc == NHC - 1),
                                     skip_group_check=True)
                nc.any.tensor_copy(out=qrow[:, ssl], in_=qp)
                warm(1)

            # e1tt fp8 cast on scalar+gpsimd (vector is the loop's co-
            # bottleneck; keep it free for e2ts and the early iterations)
            for hc in range(NHC):
                for half in range(2):
                    hsl = slice(half * XL // 2, (half + 1) * XL // 2)
                    if (2 * hc + half) % 2 == 0:
                        nc.scalar.copy(out=e1tt[:, hc, hsl],
                                       in_=e1ttb[:, hc, hsl])
                    else:
                        nc.gpsimd.tensor_copy(out=e1tt[:, hc, hsl],
                                              in_=e1ttb[:, hc, hsl])

            # packed w1..w4 [H, 4*OUTP] bf16; w2/w3 also cast fp8 for DR
            w_all = res.tile([P, NHC, 4 * OUTP], bf16, tag="w_all")
            w2q = res.tile([P, NHC, OUTP], fp8, tag="w2q")
            w3q = res.tile([P, NHC, OUTP], fp8, tag="w3q")
            for hc in range(NHC):
                nc.sync.dma_start(out=w_all[:, hc, :],
                                  in_=wall_d[hc * P:(hc + 1) * P, :])
            for hc in range(NHC):
                nc.any.tensor_copy(out=w2q[:, hc, :],
                                   in_=w_all[:, hc, OUTP:2 * OUTP])
                nc.any.tensor_copy(out=w3q[:, hc, :],
                                   in_=w_all[:, hc, 2 * OUTP:3 * OUTP])

            # ---- stats tiles ----
            M_sb = small.tile([P, NJT], f32, tag="M")
            c_sb = small.tile([P, NJT], f32, tag="c_sb")
            Z_sb = small.tile([P, NJT], f32, tag="Z")
            rZ_sb = small.tile([P, NJT], f32, tag="rZ")
            out_sb = res.tile([P, NJT, OUTP], f32, tag="out_sb")

            # all c columns up-front (PE filler during the load):
            # c^T = e2tt^T @ (16 w_c), then /16
            for jt in range(NJT):
                jsl = slice(jt * P, (jt + 1) * P)
                cp = pss.tile([P, 1], f32, tag="pss", name=f"cp{jt}")
                for hc in range(NHC):
                    nc.tensor.matmul(cp, e2tt[:, hc, jsl], wc_sb[:, hc:hc + 1],
                                     start=(hc == 0), stop=(hc == NHC - 1),
                                     skip_group_check=True)
                nc.any.tensor_scalar_mul(c_sb[:, jt:jt + 1], cp, 1.0 / WS)

            # natural-layout transposes, drip-fed into the loop as PE filler:
            # all of e1n (y2x stationary) must land before Y(0); e2n (x2y
            # stationary) is only needed in the tail.
            trans_q = ([("e1", ic) for ic in range(NIC)] +
                       [("e2", jc) for jc in range(NJT)])

            def drip_trans(n):
                for _ in range(n):
                    if not trans_q:
                        return
                    kind, ck = trans_q.pop(0)
                    src_tt = e1ttbq if kind == "e1" else e2tt
                    dst = e1n if kind == "e1" else e2n
                    for b in range(2):
                        ps = ptp.tile([P, 3, P], f32, tag="ptp",
                                      name=f"nt{kind}{ck}_{b}")
                        for k in range(3):
                            hc = 3 * b + k
                            nc.tensor.matmul(
                                ps[:, k, :],
                                src_tt[:, hc, ck * P:(ck + 1) * P],
                                ident16, start=True, stop=True,
                                skip_group_check=True)
                        nc.any.tensor_copy(
                            out=dst[ck // 2][:, ck % 2,
                                             3 * b * P:(3 * b + 3) * P],
                            in_=ps)

            # ---- main loop: software-pipelined, y2x grouped by 4 tiles ----
            sjt_cm = tc.tile_pool(name="sjt", bufs=2)
            sjt = sjt_cm.__enter__()
            sg4_cm = tc.tile_pool(name="sg4", bufs=2)
            sg4 = sg4_cm.__enter__()
            tiles = {}
            gtiles = {}

            def a_phase(jt):
                jsl = slice(jt * P, (jt + 1) * P)
                # 16*s = 16*q + (e2*16wcq) @ e1^T; u = exp(16s/16 - SHIFT)
                u = sjt.tile([P, XL], bf16, tag="u", name=f"u{jt}")
                Zp = sjt.tile([P, NSLAB], f32, tag="Zp", name=f"Zp{jt}")
                for sl in range(NSLAB):
                    ssl = slice(sl * SLAB, (sl + 1) * SLAB)
                    sp = pss.tile([P, SLAB], f32, tag="pss", name=f"sp{jt}_{sl}")
                    nc.tensor.matmul(sp, ones16, qrow[:, ssl],
                                     start=True, stop=False,
                                     skip_group_check=True)
                    for hp in range(NHC // 2):
                        nc.tensor.matmul(
                            sp, e2ts[:, 2 * hp:2 * hp + 2, jsl],
                            e1tt[:, 2 * hp:2 * hp + 2, ssl],
                            start=False, stop=(hp == NHC // 2 - 1),
                            perf_mode=DR, skip_group_check=True)
                    nc.scalar.activation(out=u[:, ssl], in_=sp, func=EXP,
                                         bias=negC, scale=1.0 / WS,
                                         accum_out=Zp[:, sl:sl + 1])
                drip_trans(4 if jt < 5 else 2)
                nc.vector.tensor_reduce(out=Z_sb[:, jt:jt + 1], in_=Zp,
                                        axis=AXX, op=ADD)
                nc.vector.reciprocal(out=rZ_sb[:, jt:jt + 1],
                                     in_=Z_sb[:, jt:jt + 1])

                # row max for b_att: M = c + SHIFT + ln(max u)
                umax = sjt.tile([P, 1], f32, tag="umax", name=f"umax{jt}")
                nc.vector.tensor_reduce(out=umax, in_=u, axis=AXX, op=MAX)
                lnu = sjt.tile([P, 1], f32, tag="lnu", name=f"lnu{jt}")
                nc.scalar.activation(out=lnu, in_=umax, func=LN)
                nc.vector.scalar_tensor_tensor(
                    out=M_sb[:, jt:jt + 1], in0=lnu, scalar=SHIFT,
                    in1=c_sb[:, jt:jt + 1], op0=ADD, op1=ADD)

                # normalized a^T transpose operand: diag(128/Z_j) -- the
                # x128 keeps small attention weights out of fp8 denormals
                diag = sjt.tile([P, P], bf16, tag="diag", name=f"diag{jt}")
                nc.vector.tensor_scalar_mul(diag, identUS, rZ_sb[:, jt:jt + 1])
                tiles[jt] = (u, diag)

            def get_uT4(g):
                if g not in gtiles:
                    gtiles[g] = sg4.tile([P, NIC, 4, P], fp8, tag="uT4",
                                         name=f"uT4_{g}")
                return gtiles[g]

            def t_phase(jt):
                u, diag = tiles.pop(jt)
                uT4 = get_uT4(jt // 4)
                jj = jt % 4
                # uT4[i, ic, jj, j] = u[j, i] * rZ_j, fp8 cast on the copy
                for g in range(NIC // 4):
                    tp = ptp.tile([P, 4, P], f32, tag="ptp", name=f"tp{jt}_{g}")
                    for k in range(4):
                        ic = g * 4 + k
                        nc.tensor.matmul(tp[:, k, :], u[:, ic * P:(ic + 1) * P],
                                         diag, start=True, stop=True,
                                         skip_group_check=True)
                    nc.any.tensor_copy(out=uT4[:, g * 4:(g + 1) * 4, jj, :],
                                       in_=tp)

            def y_phase(g):
                uT4 = gtiles.pop(g)
                gsl = slice(g * 4 * P, (g + 1) * 4 * P)
                y2xT4 = sg4.tile([P, NHC, 4 * P], fp8, tag="y2xT4",
                                 name=f"y2xT4_{g}")
                bl34 = sg4.tile([P, NHC, 4 * P], fp8, tag="bl34",
                                name=f"bl34_{g}")
                for hc in range(NHC):
                    yp = psy.tile([P, 4 * P], f32, tag="psy", name=f"yp{g}_{hc}")
                    for icp in range(NIC // 2):
                        nc.tensor.matmul(
                            yp,
                            e1n[icp][:, :, hc * P:(hc + 1) * P],
                            uT4[:, 2 * icp:2 * icp + 2, :, :],
                            start=(icp == 0), stop=(icp == NIC // 2 - 1),
                            perf_mode=DR, skip_group_check=True)
                    nc.vector.tensor_scalar_mul(y2xT4[:, hc, :], yp,
                                                 1.0 / 128.0)
                    nc.vector.tensor_mul(bl34[:, hc, :], e2tt[:, hc, gsl],
                                         y2xT4[:, hc, :])
                gtiles[(g, "y")] = (y2xT4, bl34)

            def c_phase(jt):
                g, jj = jt // 4, jt % 4
                y2xT4, bl34 = gtiles[(g, "y")]
                jsl4 = slice(jj * P, (jj + 1) * P)
                # pass-1 reduction: [y2x; e2*y2x] @ 16*[w2; w3] (DoubleRow)
                op1 = pso.tile([P, OUTP], f32, tag="pso", name=f"op1_{jt}")
                for hp in range(NHC // 2):
                    nc.tensor.matmul(op1, y2xT4[:, 2 * hp:2 * hp + 2, jsl4],
                                     w2q[:, 2 * hp:2 * hp + 2, :],
                                     start=(hp == 0), stop=False,
                                     perf_mode=DR, skip_group_check=True)
                for hp in range(NHC // 2):
                    nc.tensor.matmul(op1, bl34[:, 2 * hp:2 * hp + 2, jsl4],
                                     w3q[:, 2 * hp:2 * hp + 2, :],
                                     start=False, stop=(hp == NHC // 2 - 1),
                                     perf_mode=DR, skip_group_check=True)
                # out_sb = psum/16 + b_red
                nc.vector.scalar_tensor_tensor(
                    out=out_sb[:, jt, :], in0=op1, scalar=1.0 / WS,
                    in1=bred_bc, op0=MUL, op1=ADD)
                if jj == 3:
                    gtiles.pop((g, "y"))

            # prologue: A0..A4 interleaved with T0..T3
            a_phase(0)
            for jt in range(1, 5):
                a_phase(jt)
                t_phase(jt - 1)
            for g in range(4):
                y_phase(g)
                if g < 3:
                    for jj in range(4):
                        jt = 4 * (g + 1) + jj
                        if jt + 1 < NJT:
                            a_phase(jt + 1)
                        t_phase(jt)
                for jj in range(4):
                    c_phase(4 * g + jj)

            sg4_cm.__exit__(None, None, None)
            sjt_cm.__exit__(None, None, None)
            # bridge the serial b_att chain so the HAM clock stays warm
            warm(6)
            post_cm = tc.tile_pool(name="post", bufs=1)
            post = post_cm.__enter__()

            # ---- b_att = softmax_j(M), no max shift (M <= ~14, f32 exp ok) ----
            bexp = post.tile([P, NJT], f32, tag="bexp")
            brow = post.tile([P, 1], f32, tag="brow")
            nc.scalar.activation(out=bexp, in_=M_sb, func=EXP, accum_out=brow)
            tpb = pss.tile([1, P], f32, tag="pss", name="tpb")
            nc.tensor.transpose(tpb, brow, ident32)
            brw = post.tile([1, P], f32, tag="brw")
            nc.vector.tensor_copy(out=brw, in_=tpb)
            bs0 = post.tile([1, 1], f32, tag="bs0")
            nc.vector.tensor_reduce(out=bs0, in_=brw, axis=AXX, op=ADD)
            rb0 = post.tile([1, 1], f32, tag="rb0")
            nc.vector.reciprocal(rb0, bs0)
            # broadcast 64/sum(b) to all partitions with a K=1 matmul
            rbp = pss.tile([P, 1], f32, tag="pss", name="rbp")
            nc.tensor.matmul(rbp, onesBS, rb0, start=True, stop=True,
                             skip_group_check=True)
            rbz = post.tile([P, 1], f32, tag="rbz")
            nc.vector.tensor_copy(out=rbz, in_=rbp)
            battq = post.tile([P, NJT], fp8, tag="battq")
            nc.vector.tensor_scalar_mul(battq, bexp, rbz)

            # x2y^T directly: x2yT[h] = sum_j e2n[j,h] * (64 b_j), then /64
            x2p = psy.tile([P, NHC], f32, tag="psy", name="x2p")
            for hc in range(NHC):
                for jc in range(NJT):
                    nc.tensor.matmul(
                        x2p[:, hc:hc + 1],
                        e2n[jc // 2][:, jc % 2, hc * P:(hc + 1) * P],
                        battq[:, jc:jc + 1],
                        start=(jc == 0), stop=(jc == NJT - 1),
                        skip_group_check=True)
                warm(1)
            x2yT = post.tile([P, NHC], f32, tag="x2yT")
            nc.vector.tensor_scalar_mul(x2yT, x2p, 1.0 / BS)

            # w14 = 16*(w1 + x2y*w4), bf16 (fp8 here costs ~3% output error)
            w14 = res.tile([P, NHC, OUTP], bf16, tag="w14")
            for hc in range(NHC):
                nc.vector.scalar_tensor_tensor(
                    out=w14[:, hc, :], in0=w_all[:, hc, 3 * OUTP:4 * OUTP],
                    scalar=x2yT[:, hc:hc + 1], in1=w_all[:, hc, 0:OUTP],
                    op0=MUL, op1=ADD)
            warm(4)

            # ---- pass 2: out += emb2 @ w14/16 (bf16), stream out ----
            for jt in range(NJT):
                jsl = slice(jt * P, (jt + 1) * P)
                op2 = pso.tile([P, OUTP], f32, tag="pso", name=f"op2_{jt}")
                for hc in range(NHC):
                    nc.tensor.matmul(op2, e2tt[:, hc, jsl], w14[:, hc, :],
                                     start=(hc == 0), stop=(hc == NHC - 1),
                                     skip_group_check=True)
                fin = stage.tile([P, OUTP], bf16, tag="fin", name=f"fin{jt}")
                nc.vector.scalar_tensor_tensor(
                    out=fin, in0=op2, scalar=1.0 / WS,
                    in1=out_sb[:, jt, :], op0=MUL, op1=ADD)
                nc.sync.dma_start(out=out_d[jsl, :], in_=fin[:, 0:OUT])
            post_cm.__exit__(None, None, None)

    return nc


def _get_nc(drain_fix=True):
    if "nc" not in _CACHE:
        _CACHE["nc"] = _build()
    if drain_fix and not _CACHE.get("drain_fixed"):
        import concourse.mybir as mybir
        _fix_waits(_CACHE["nc"], mybir, max_waits=1)
        _CACHE["drain_fixed"] = True
    return _CACHE["nc"]


def _prep_weights(w_c, w_q, w_cq, w_red, b_red):
    bf = ml_dtypes.bfloat16
    w_red = np.asarray(w_red, dtype=np.float32)

    wall = np.zeros((H, 4 * OUTP), np.float32)
    for k in range(4):
        wall[:, k * OUTP:k * OUTP + OUT] = w_red[k * H:(k + 1) * H]
    bredp = np.zeros((1, OUTP), np.float32)
    bredp[0, :OUT] = np.asarray(b_red, np.float32)
    return {
        "wc": np.ascontiguousarray(
            (np.asarray(w_c, np.float32) * WS).reshape(NHC, P).T.astype(bf)),
        "wq": np.ascontiguousarray(
            (np.asarray(w_q, np.float32) * WS).reshape(NHC, P).T.astype(bf)),
        "wcq": np.ascontiguousarray(
            (np.asarray(w_cq, np.float32) * WS).reshape(NHC, P).T),
        "wall": np.ascontiguousarray((wall * WS).astype(bf)),
        "bred": np.ascontiguousarray(bredp),
    }


def kernel(emb1, emb2, w_c, b_c, w_q, b_q, w_cq, b_cq, w_red, b_red):
    from concourse.bass_utils import run_bass_kernel_spmd

    nc = _get_nc()
    bf = ml_dtypes.bfloat16

    # host-side transpose: DMA rows become 4KB (packet-rate-limited queues)
    emb1t = np.ascontiguousarray(
        np.asarray(emb1, dtype=np.float32).transpose(0, 2, 1).astype(bf))
    emb2t = np.ascontiguousarray(
        np.asarray(emb2, dtype=np.float32).transpose(0, 2, 1).astype(bf))

    # b_c, b_q, b_cq cancel exactly in both softmaxes (per-row/col consts).
    prep = _prep_weights(w_c, w_q, w_cq, w_red, b_red)

    in_maps = []
    for b in range(NCORES):
        in_maps.append({"emb1t": emb1t[b], "emb2t": emb2t[b], **prep})
    res = run_bass_kernel_spmd(nc, in_maps, core_ids=list(range(NCORES)))
    return np.stack([res.results[i]["out"] for i in range(NCORES)],
                    axis=0).astype(np.float32)


# revision 31
# speedup vs baseline: 1.4040x; 1.3848x over previous
"""BiDAF attention-flow kernel for Trainium2 (8 NeuronCores, data-parallel over batch).

Per core (one batch element):
  s[j,i]   = c[j] + q[i] + sum_h w_cq[h]*emb2[j,h]*emb1[i,h]
  a        = softmax_i(s)          (c[j] drops out of the row softmax)
  y2x      = a @ emb1
  b_att    = softmax_j(max_i s)
  x2y      = sum_j b_att[j]*emb2[j]
  out      = [emb2, y2x, emb2*y2x, emb2*x2y] @ w_red + b_red

Implementation notes:
  - b_c/b_q/b_cq cancel exactly in both softmaxes (row/column constants).
  - Row softmax uses a FIXED exp shift (s - SHIFT); true row max recovered as
    SHIFT + ln(max_i u) for b_att.
  - The s-matmul, y2x and pass-1 run on fp8(e4m3) with DoubleRow perf mode
    (2 K-planes per instruction, ~1.8x over bf16 at N=512).  q, c, pass-2 and
    u stay bf16: fp8 there costs ~1-3% output error for little speed.
    Weights are pre-scaled x16 on the host so their fp8 encodings stay in the
    normal range; the 1/16 descale is folded into the exp scale / output STTs.
  - Softmax normalization is folded into the PE transpose of u: transposing
    against diag(1/Z_j) instead of the identity yields normalized a^T free.
  - y2x is computed per GROUP of 4 j-tiles as N=512 DoubleRow sweeps (DR needs
    FD>=256 to beat fast-weight-load bf16).
  - Embeddings arrive TRANSPOSED from the host ([H, L]): DMA rows are then
    4KB (vs 1.5KB), ~2.7x fewer descriptors on the packet-rate-limited DMA
    queues, and e2^T needs no on-device transposing at all.  w1..w4 travel as
    one packed [H, 4*OUTP] tensor for the same reason.  Natural-layout copies
    (e1n for y2x, e2n for x2y) are re-derived by PE transposes, which are
    REGULAR matmuls against an identity: transpose-mode does not count as PE
    activity for the HAM clock gate, real matmuls do, so the load phase warms
    the clock to 2.4 GHz and in-loop e2n transposes keep it there.
  - Main loop is software-pipelined (A = s/exp/stats, T = u transposes, Y =
    grouped y2x, C = pass-1) so the in-order PE queue never head-blocks on a
    fresh dependency; idle >3.4us would re-throttle the clock to 1.2 GHz.
"""

import numpy as np
import ml_dtypes

P = 128
XL = 2048
YL = 2048
H = 768
OUT = 300
OUTP = 320      # OUT padded to a 16B-aligned fp8 stride for DoubleRow
NJT = YL // P   # 16 j tiles
NIC = XL // P   # 16 i chunks
NHC = H // P    # 6 h chunks
SLAB = 512
NSLAB = XL // SLAB  # 4
NCORES = 8
SHIFT = 2.0     # fixed exp shift; keeps u = exp(s-SHIFT) in fp8/bf16 range
WS = 16.0       # host-side weight scale (wq, wc, wcq, w1..w4)
BS = 64.0       # b_att fp8 scale

_CACHE = {}


def _fix_waits(nc, mybir, max_waits=1):
    """This walrus build rejects >1 sync wait per instruction.

    Pass 1: drop waits that are transitively implied by another wait on the
    same instruction.  Pass 2: hoist remaining extra waits onto same-engine
    NoOps inserted right before the instruction.
    """
    from collections import defaultdict

    blocks = [bb for f in nc.m.functions for bb in f.blocks]
    insts = [ins for bb in blocks for ins in bb.instructions]

    dma_types = ("InstDMACopy", "InstDmaTransposeAnt")
    eng_stream = defaultdict(list)
    queue_stream = defaultdict(list)
    sem_events = defaultdict(list)
    cum = defaultdict(int)
    for i, ins in enumerate(insts):
        eng_stream[str(ins.engine)].append(i)
        si = ins.sync_info
        if si and si.on_update:
            for u in si.on_update:
                cum[u.id] += u.update_value
                sem_events[u.id].append((cum[u.id], i))
                if type(ins).__name__ in dma_types:
                    queue_stream[u.id].append(i)

    def achiever(sem_id, val):
        for cv, i in sem_events.get(sem_id, []):
            if cv >= val:
                return i
        return None

    eng_pos, q_pos = {}, {}
    for e, lst in eng_stream.items():
        for k, i in enumerate(lst):
            eng_pos[i] = (e, k)
    for s, lst in queue_stream.items():
        for k, i in enumerate(lst):
            q_pos[i] = (s, k)

    memo = {}

    def implied(i):
        if i in memo:
            return memo[i]
        memo[i] = set()
        out = {i}
        ins = insts[i]
        if i in q_pos:
            s, k = q_pos[i]
            if k > 0:
                out |= implied(queue_stream[s][k - 1])
        e, k = eng_pos[i]
        j = k - 1
        while j >= 0:
            p = eng_stream[e][j]
            if type(insts[p]).__name__ in dma_types:
                j -= 1
                continue
            out |= implied(p)
            break
        si = ins.sync_info
        if si and si.on_wait:
            for w in si.on_wait:
                a = achiever(w.id, w.wait_value)
                if a is not None:
                    out |= implied(a)
        memo[i] = out
        return out

    # pass 1: redundancy elimination
    for i, ins in enumerate(insts):
        si = ins.sync_info
        if not (si and si.on_wait and len(si.on_wait) > max_waits):
            continue
        waits = list(si.on_wait)
        ach = [(w, achiever(w.id, w.wait_value)) for w in waits]
        keep = []
        for wi, (w, a) in enumerate(ach):
            red = False
            if a is not None:
                for wj, (w2, a2) in enumerate(ach):
                    if wi != wj and a2 is not None and a != a2 and a in implied(a2):
                        red = True
                        break
            if not red:
                keep.append(w)
        si.on_wait = keep

    # pass 2: hoist extras onto same-engine NoOps
    k = 0
    for bb in blocks:
        lst = bb.instructions
        i = 0
        while i < len(lst):
            ins = lst[i]
            si = ins.sync_info
            if si and si.on_wait and len(si.on_wait) > max_waits:
                waits = list(si.on_wait)
                extra, keep = waits[:-max_waits], waits[-max_waits:]
                si.on_wait = keep
                nops = []
                for w in extra:
                    nop = mybir.InstNoOp(name=f"I-waitfix-{k}", ins=[], outs=[])
                    k += 1
                    nop.engine = ins.engine
                    nop.sync_info = mybir.SyncInfo(on_wait=[w], on_update=[])
                    nops.append(nop)
                lst[i:i] = nops
                i += len(nops)
            i += 1


def _build():
    import concourse.bass as bass
    import concourse.tile as tile
    import concourse.mybir as mybir
    from concourse.masks import make_identity

    f32 = mybir.dt.float32
    bf16 = mybir.dt.bfloat16
    fp8 = mybir.dt.float8e4
    DR = mybir.MatmulPerfMode.DoubleRow
    MUL = mybir.AluOpType.mult
    ADD = mybir.AluOpType.add
    MAX = mybir.AluOpType.max
    EXP = mybir.ActivationFunctionType.Exp
    LN = mybir.ActivationFunctionType.Ln
    AXX = mybir.AxisListType.X

    nc = bass.Bass("TRN2", target_bir_lowering=False, debug=False,
                   num_devices=NCORES)

    emb1t_d = nc.dram_tensor("emb1t", [H, XL], bf16, kind="ExternalInput")
    emb2t_d = nc.dram_tensor("emb2t", [H, YL], bf16, kind="ExternalInput")
    wc_d = nc.dram_tensor("wc", [P, NHC], bf16, kind="ExternalInput")
    wq_d = nc.dram_tensor("wq", [P, NHC], bf16, kind="ExternalInput")
    wcq_d = nc.dram_tensor("wcq", [P, NHC], f32, kind="ExternalInput")
    wall_d = nc.dram_tensor("wall", [H, 4 * OUTP], bf16, kind="ExternalInput")
    bred_d = nc.dram_tensor("bred", [1, OUTP], f32, kind="ExternalInput")
    out_d = nc.dram_tensor("out", [YL, OUT], bf16, kind="ExternalOutput")

    with tile.TileContext(nc) as tc:
        with (
            tc.tile_pool(name="res", bufs=1) as res,        # resident data
            tc.tile_pool(name="stage", bufs=3) as stage,    # out staging
            tc.tile_pool(name="small", bufs=1) as small,    # stats etc
            tc.tile_pool(name="pss", bufs=2, space="PSUM") as pss,
            tc.tile_pool(name="ptp", bufs=2, space="PSUM") as ptp,
            tc.tile_pool(name="psy", bufs=2, space="PSUM") as psy,
            tc.tile_pool(name="pso", bufs=2, space="PSUM") as pso,
        ):
            # ---- constants ----
            ident16 = res.tile([P, P], bf16, tag="ident16")
            make_identity(nc, ident16)
            ident32 = res.tile([P, P], f32, tag="ident32")
            make_identity(nc, ident32)
            onesBS = res.tile([1, P], f32, tag="onesBS")
            nc.vector.memset(onesBS, BS)
            ones16 = res.tile([1, P], bf16, tag="ones16")
            nc.vector.memset(ones16, 1.0)
            identUS = res.tile([P, P], bf16, tag="identUS")
            nc.vector.tensor_scalar_mul(identUS, ident16, 128.0)
            negC = res.tile([P, 1], f32, tag="negC")
            nc.vector.memset(negC, -SHIFT)
            warm_sb = res.tile([P, OUTP], bf16, tag="warm_sb")
            nc.vector.memset(warm_sb, 0.0)

            # PE warm-up with REAL matmuls: HAM grants 2.4 GHz only after
            # ~3.4us of sustained activity; these bridge t=0 until the load
            # transposes / q matmuls take over.
            _warm_k = [0]

            def warm(n):
                for _ in range(n):
                    wk = _warm_k[0]
                    _warm_k[0] += 1
                    wps = pso.tile([P, OUTP], f32, tag="pso", name=f"warm{wk}")
                    nc.tensor.matmul(wps, ident16, warm_sb, start=True,
                                     stop=True, skip_group_check=True)

            warm(12)

            # ---- small weights ----
            wc_sb = res.tile([P, NHC], bf16, tag="wc")
            nc.sync.dma_start(out=wc_sb, in_=wc_d[:])
            wq_sb = res.tile([P, NHC], bf16, tag="wq")
            nc.sync.dma_start(out=wq_sb, in_=wq_d[:])
            wcq_sb = res.tile([P, NHC], f32, tag="wcq")
            nc.sync.dma_start(out=wcq_sb, in_=wcq_d[:])
            bred_bc = res.tile([P, OUTP], f32, tag="bred_bc")
            _bap = bred_d.ap()
            nc.sync.dma_start(out=bred_bc, in_=bass.AP(
                tensor=_bap.tensor, offset=_bap.offset,
                ap=[[0, P]] + list(_bap.ap[1:])))

            # ---- transposed embeddings: direct DMA, 4KB rows ----
            # e2tt bf16 resident; e2ts = e2tt * (16 w_cq) fp8; e1tt fp8.
            # e1n (fp8 natural pairs, y2x stationary) via PE transposes.
            # e2n (fp8 natural pairs, x2y stationary) transposed in-loop.
            e2tt = res.tile([P, NHC, YL], bf16, tag="e2tt")
            e2ts = res.tile([P, NHC, YL], fp8, tag="e2ts")
            e1tt = res.tile([P, NHC, XL], fp8, tag="e1tt")
            e1n = [res.tile([P, 2, H], fp8, tag=f"e1n{i}", name=f"e1n{i}")
                   for i in range(NIC // 2)]
            e2n = [res.tile([P, 2, H], fp8, tag=f"e2n{i}", name=f"e2n{i}")
                   for i in range(NJT // 2)]
            e1ttbq = res.tile([P, NHC, XL], bf16, tag="e1ttb")
            e1ttb = e1ttbq

            for hc in range(NHC):
                nc.sync.dma_start(out=e2tt[:, hc, :],
                                  in_=emb2t_d[hc * P:(hc + 1) * P, :])
                nc.vector.tensor_scalar_mul(
                    e2ts[:, hc, :], e2tt[:, hc, :], wcq_sb[:, hc:hc + 1])
            for hc in range(NHC):
                nc.sync.dma_start(out=e1ttb[:, hc, :],
                                  in_=emb1t_d[hc * P:(hc + 1) * P, :])

            # q row (bf16): q^T(x16) = (16 w_q)^T @ emb1^T
            qrow = small.tile([1, XL], bf16, tag="qrow")
            for sl in range(NSLAB):
                ssl = slice(sl * SLAB, (sl + 1) * SLAB)
                qp = pss.tile([1, SLAB], f32, tag="pss", name=f"qp{sl}")
                for hc in range(NHC):
                    nc.tensor.matmul(qp, wq_sb[:, hc:hc + 1],
                                     e1ttb[:, hc, ssl],
                                     start=(hc == 0), stop=(hc == NHC - 1),
                                     skip_group_check=True)
                nc.any.tensor_copy(out=qrow[:, ssl], in_=qp)
                warm(1)

            # e1tt fp8 cast on scalar (2/3) + vector (1/3); gpsimd measures
            # ~10x below spec on big tensor ops -- never give it bulk work
            for hc in range(NHC):
                for half in range(2):
                    hsl = slice(half * XL // 2, (half + 1) * XL // 2)
                    if (2 * hc + half) % 3 == 2:
                        nc.vector.tensor_copy(out=e1tt[:, hc, hsl],
                                              in_=e1ttb[:, hc, hsl])
                    else:
                        nc.scalar.copy(out=e1tt[:, hc, hsl],
                                       in_=e1ttb[:, hc, hsl])

            # packed w1..w4 [H, 4*OUTP] bf16; w2/w3 also cast fp8 for DR
            w_all = res.tile([P, NHC, 4 * OUTP], bf16, tag="w_all")
            w2q = res.tile([P, NHC, OUTP], fp8, tag="w2q")
            w3q = res.tile([P, NHC, OUTP], fp8, tag="w3q")
            for hc in range(NHC):
                nc.sync.dma_start(out=w_all[:, hc, :],
                                  in_=wall_d[hc * P:(hc + 1) * P, :])
            for hc in range(NHC):
                nc.any.tensor_copy(out=w2q[:, hc, :],
                                   in_=w_all[:, hc, OUTP:2 * OUTP])
                nc.any.tensor_copy(out=w3q[:, hc, :],
                                   in_=w_all[:, hc, 2 * OUTP:3 * OUTP])

            # ---- stats tiles ----
            M_sb = small.tile([P, NJT], f32, tag="M")
            c_sb = small.tile([P, NJT], f32, tag="c_sb")
            Z_sb = small.tile([P, NJT], f32, tag="Z")
            rZ_sb = small.tile([P, NJT], f32, tag="rZ")
            out_sb = res.tile([P, NJT, OUTP], f32, tag="out_sb")

            # all c columns up-front (PE filler during the load):
            # c^T = e2tt^T @ (16 w_c), then /16
            for jt in range(NJT):
                jsl = slice(jt * P, (jt + 1) * P)
                cp = pss.tile([P, 1], f32, tag="pss", name=f"cp{jt}")
                for hc in range(NHC):
                    nc.tensor.matmul(cp, e2tt[:, hc, jsl], wc_sb[:, hc:hc + 1],
                                     start=(hc == 0), stop=(hc == NHC - 1),
                                     skip_group_check=True)
                nc.any.tensor_scalar_mul(c_sb[:, jt:jt + 1], cp, 1.0 / WS)

            # natural-layout transposes, drip-fed into the loop as PE filler:
            # all of e1n (y2x stationary) must land before Y(0); e2n (x2y
            # stationary) is only needed in the tail.
            trans_q = ([("e1", ic) for ic in range(NIC)] +
                       [("e2", jc) for jc in range(NJT)])

            def drip_trans(n):
                for _ in range(n):
                    if not trans_q:
                        return
                    kind, ck = trans_q.pop(0)
                    src_tt = e1ttbq if kind == "e1" else e2tt
                    dst = e1n if kind == "e1" else e2n
                    for b in range(2):
                        ps = ptp.tile([P, 3, P], f32, tag="ptp",
                                      name=f"nt{kind}{ck}_{b}")
                        for k in range(3):
                            hc = 3 * b + k
                            nc.tensor.matmul(
                                ps[:, k, :],
                                src_tt[:, hc, ck * P:(ck + 1) * P],
                                ident16, start=True, stop=True,
                                skip_group_check=True)
                        nc.any.tensor_copy(
                            out=dst[ck // 2][:, ck % 2,
                                             3 * b * P:(3 * b + 3) * P],
                            in_=ps)

            # ---- main loop: software-pipelined, y2x grouped by 4 tiles ----
            sjt_cm = tc.tile_pool(name="sjt", bufs=2)
            sjt = sjt_cm.__enter__()
            sg4_cm = tc.tile_pool(name="sg4", bufs=2)
            sg4 = sg4_cm.__enter__()
            tiles = {}
            gtiles = {}

            def a_phase(jt):
                jsl = slice(jt * P, (jt + 1) * P)
                # 16*s = 16*q + (e2*16wcq) @ e1^T; u = exp(16s/16 - SHIFT)
                u = sjt.tile([P, XL], bf16, tag="u", name=f"u{jt}")
                Zp = sjt.tile([P, NSLAB], f32, tag="Zp", name=f"Zp{jt}")
                for sl in range(NSLAB):
                    ssl = slice(sl * SLAB, (sl + 1) * SLAB)
                    sp = pss.tile([P, SLAB], f32, tag="pss", name=f"sp{jt}_{sl}")
                    nc.tensor.matmul(sp, ones16, qrow[:, ssl],
                                     start=True, stop=False,
                                     skip_group_check=True)
                    for hp in range(NHC // 2):
                        nc.tensor.matmul(
                            sp, e2ts[:, 2 * hp:2 * hp + 2, jsl],
                            e1tt[:, 2 * hp:2 * hp + 2, ssl],
                            start=False, stop=(hp == NHC // 2 - 1),
                            perf_mode=DR, skip_group_check=True)
                    nc.scalar.activation(out=u[:, ssl], in_=sp, func=EXP,
                                         bias=negC, scale=1.0 / WS,
                                         accum_out=Zp[:, sl:sl + 1])
                drip_trans(4 if jt < 5 else 2)
                nc.vector.tensor_reduce(out=Z_sb[:, jt:jt + 1], in_=Zp,
                                        axis=AXX, op=ADD)
                nc.vector.reciprocal(out=rZ_sb[:, jt:jt + 1],
                                     in_=Z_sb[:, jt:jt + 1])

                # row max for b_att: M = c + SHIFT + ln(max u)
                umax = sjt.tile([P, 1], f32, tag="umax", name=f"umax{jt}")
                nc.vector.tensor_reduce(out=umax, in_=u, axis=AXX, op=MAX)
                lnu = sjt.tile([P, 1], f32, tag="lnu", name=f"lnu{jt}")
                nc.scalar.activation(out=lnu, in_=umax, func=LN)
                nc.vector.scalar_tensor_tensor(
                    out=M_sb[:, jt:jt + 1], in0=lnu, scalar=SHIFT,
                    in1=c_sb[:, jt:jt + 1], op0=ADD, op1=ADD)

                # normalized a^T transpose operand: diag(128/Z_j) -- the
                # x128 keeps small attention weights out of fp8 denormals
                diag = sjt.tile([P, P], bf16, tag="diag", name=f"diag{jt}")
                nc.vector.tensor_scalar_mul(diag, identUS, rZ_sb[:, jt:jt + 1])
                tiles[jt] = (u, diag)

            def get_uT4(g):
                if g not in gtiles:
                    gtiles[g] = sg4.tile([P, NIC, 4, P], fp8, tag="uT4",
                                         name=f"uT4_{g}")
                return gtiles[g]

            def t_phase(jt):
                u, diag = tiles.pop(jt)
                uT4 = get_uT4(jt // 4)
                jj = jt % 4
                # uT4[i, ic, jj, j] = u[j, i] * rZ_j, fp8 cast on the copy
                for g in range(NIC // 4):
                    tp = ptp.tile([P, 4, P], f32, tag="ptp", name=f"tp{jt}_{g}")
                    for k in range(4):
                        ic = g * 4 + k
                        nc.tensor.matmul(tp[:, k, :], u[:, ic * P:(ic + 1) * P],
                                         diag, start=True, stop=True,
                                         skip_group_check=True)
                    nc.any.tensor_copy(out=uT4[:, g * 4:(g + 1) * 4, jj, :],
                                       in_=tp)

            def y_phase(g):
                uT4 = gtiles.pop(g)
                gsl = slice(g * 4 * P, (g + 1) * 4 * P)
                y2xT4 = sg4.tile([P, NHC, 4 * P], fp8, tag="y2xT4",
                                 name=f"y2xT4_{g}")
                bl34 = sg4.tile([P, NHC, 4 * P], fp8, tag="bl34",
                                name=f"bl34_{g}")
                for hc in range(NHC):
                    yp = psy.tile([P, 4 * P], f32, tag="psy", name=f"yp{g}_{hc}")
                    for icp in range(NIC // 2):
                        nc.tensor.matmul(
                            yp,
                            e1n[icp][:, :, hc * P:(hc + 1) * P],
                            uT4[:, 2 * icp:2 * icp + 2, :, :],
                            start=(icp == 0), stop=(icp == NIC // 2 - 1),
                            perf_mode=DR, skip_group_check=True)
                    nc.vector.tensor_scalar_mul(y2xT4[:, hc, :], yp,
                                                 1.0 / 128.0)
                    nc.vector.tensor_mul(bl34[:, hc, :], e2tt[:, hc, gsl],
                                         y2xT4[:, hc, :])
                gtiles[(g, "y")] = (y2xT4, bl34)

            def c_phase(jt):
                g, jj = jt // 4, jt % 4
                y2xT4, bl34 = gtiles[(g, "y")]
                jsl4 = slice(jj * P, (jj + 1) * P)
                # pass-1 reduction: [y2x; e2*y2x] @ 16*[w2; w3] (DoubleRow)
                op1 = pso.tile([P, OUTP], f32, tag="pso", name=f"op1_{jt}")
                for hp in range(NHC // 2):
                    nc.tensor.matmul(op1, y2xT4[:, 2 * hp:2 * hp + 2, jsl4],
                                     w2q[:, 2 * hp:2 * hp + 2, :],
                                     start=(hp == 0), stop=False,
                                     perf_mode=DR, skip_group_check=True)
                for hp in range(NHC // 2):
                    nc.tensor.matmul(op1, bl34[:, 2 * hp:2 * hp + 2, jsl4],
                                     w3q[:, 2 * hp:2 * hp + 2, :],
                                     start=False, stop=(hp == NHC // 2 - 1),
                                     perf_mode=DR, skip_group_check=True)
                # out_sb = psum/16 + b_red
                nc.vector.scalar_tensor_tensor(
                    out=out_sb[:, jt, :], in0=op1, scalar=1.0 / WS,
                    in1=bred_bc, op0=MUL, op1=ADD)
                if jj == 3:
                    gtiles.pop((g, "y"))

            # prologue: A0..A4 interleaved with T0..T3
            a_phase(0)
            for jt in range(1, 5):
                a_phase(jt)
                t_phase(jt - 1)
            post_cm = tc.tile_pool(name="post", bufs=1)
            post = post_cm.__enter__()
            for g in range(4):
                y_phase(g)
                if g < 3:
                    for jj in range(4):
                        jt = 4 * (g + 1) + jj
                        if jt + 1 < NJT:
                            a_phase(jt + 1)
                        t_phase(jt)
                    for jj in range(4):
                        c_phase(4 * g + jj)
                else:
                    # epilogue: b_att chain + x2y overlap the last c-phases
                    # ---- b_att = softmax_j(M), no max shift ----
                    bexp = post.tile([P, NJT], f32, tag="bexp")
                    brow = post.tile([P, 1], f32, tag="brow")
                    nc.scalar.activation(out=bexp, in_=M_sb, func=EXP,
                                         accum_out=brow)
                    tpb = pss.tile([1, P], f32, tag="pss", name="tpb")
                    nc.tensor.transpose(tpb, brow, ident32)
                    brw = post.tile([1, P], f32, tag="brw")
                    nc.vector.tensor_copy(out=brw, in_=tpb)
                    bs0 = post.tile([1, 1], f32, tag="bs0")
                    nc.vector.tensor_reduce(out=bs0, in_=brw, axis=AXX, op=ADD)
                    rb0 = post.tile([1, 1], f32, tag="rb0")
                    nc.vector.reciprocal(rb0, bs0)
                    rbp = pss.tile([P, 1], f32, tag="pss", name="rbp")
                    nc.tensor.matmul(rbp, onesBS, rb0, start=True, stop=True,
                                     skip_group_check=True)
                    rbz = post.tile([P, 1], f32, tag="rbz")
                    nc.vector.tensor_copy(out=rbz, in_=rbp)
                    battq = post.tile([P, NJT], fp8, tag="battq")
                    nc.vector.tensor_scalar_mul(battq, bexp, rbz)

                    c_phase(12)
                    c_phase(13)

                    # x2y^T: x2yT[h] = sum_j e2n[j,h] * (64 b_j), then /64
                    x2p = psy.tile([P, NHC], f32, tag="psy", name="x2p")
                    for hc in range(NHC):
                        for jc in range(NJT):
                            nc.tensor.matmul(
                                x2p[:, hc:hc + 1],
                                e2n[jc // 2][:, jc % 2, hc * P:(hc + 1) * P],
                                battq[:, jc:jc + 1],
                                start=(jc == 0), stop=(jc == NJT - 1),
                                skip_group_check=True)
                    c_phase(14)
                    c_phase(15)
                    x2yT = post.tile([P, NHC], f32, tag="x2yT")
                    nc.vector.tensor_scalar_mul(x2yT, x2p, 1.0 / BS)

                    # w14 = 16*(w1 + x2y*w4), bf16
                    w14 = res.tile([P, NHC, OUTP], bf16, tag="w14")
                    for hc in range(NHC):
                        nc.vector.scalar_tensor_tensor(
                            out=w14[:, hc, :],
                            in0=w_all[:, hc, 3 * OUTP:4 * OUTP],
                            scalar=x2yT[:, hc:hc + 1],
                            in1=w_all[:, hc, 0:OUTP],
                            op0=MUL, op1=ADD)
                    warm(6)

            post_cm.__exit__(None, None, None)
            sg4_cm.__exit__(None, None, None)
            sjt_cm.__exit__(None, None, None)

            # ---- pass 2: out += emb2 @ w14/16 (bf16), stream out ----
            for jt in range(NJT):
                jsl = slice(jt * P, (jt + 1) * P)
                op2 = pso.tile([P, OUTP], f32, tag="pso", name=f"op2_{jt}")
                for hc in range(NHC):
                    nc.tensor.matmul(op2, e2tt[:, hc, jsl], w14[:, hc, :],
                                     start=(hc == 0), stop=(hc == NHC - 1),
                                     skip_group_check=True)
                fin = stage.tile([P, OUTP], bf16, tag="fin", name=f"fin{jt}")
                nc.vector.scalar_tensor_tensor(
                    out=fin, in0=op2, scalar=1.0 / WS,
                    in1=out_sb[:, jt, :], op0=MUL, op1=ADD)
                nc.sync.dma_start(out=out_d[jsl, :], in_=fin[:, 0:OUT])

    return nc


def _get_nc(drain_fix=True):
    if "nc" not in _CACHE:
        _CACHE["nc"] = _build()
    if drain_fix and not _CACHE.get("drain_fixed"):
        import concourse.mybir as mybir
        _fix_waits(_CACHE["nc"], mybir, max_waits=1)
        _CACHE["drain_fixed"] = True
    return _CACHE["nc"]


def _prep_weights(w_c, w_q, w_cq, w_red, b_red):
    bf = ml_dtypes.bfloat16
    w_red = np.asarray(w_red, dtype=np.float32)

    wall = np.zeros((H, 4 * OUTP), np.float32)
    for k in range(4):
        wall[:, k * OUTP:k * OUTP + OUT] = w_red[k * H:(k + 1) * H]
    bredp = np.zeros((1, OUTP), np.float32)
    bredp[0, :OUT] = np.asarray(b_red, np.float32)
    return {
        "wc": np.ascontiguousarray(
            (np.asarray(w_c, np.float32) * WS).reshape(NHC, P).T.astype(bf)),
        "wq": np.ascontiguousarray(
            (np.asarray(w_q, np.float32) * WS).reshape(NHC, P).T.astype(bf)),
        "wcq": np.ascontiguousarray(
            (np.asarray(w_cq, np.float32) * WS).reshape(NHC, P).T),
        "wall": np.ascontiguousarray((wall * WS).astype(bf)),
        "bred": np.ascontiguousarray(bredp),
    }


def kernel(emb1, emb2, w_c, b_c, w_q, b_q, w_cq, b_cq, w_red, b_red):
    from concourse.bass_utils import run_bass_kernel_spmd

    nc = _get_nc()
    bf = ml_dtypes.bfloat16

    # host-side transpose: DMA rows become 4KB (packet-rate-limited queues)
    emb1t = np.ascontiguousarray(
        np.asarray(emb1, dtype=np.float32).transpose(0, 2, 1).astype(bf))
    emb2t = np.ascontiguousarray(
        np.asarray(emb2, dtype=np.float32).transpose(0, 2, 1).astype(bf))

    # b_c, b_q, b_cq cancel exactly in both softmaxes (per-row/col consts).
    prep = _prep_weights(w_c, w_q, w_cq, w_red, b_red)

    in_maps = []
    for b in range(NCORES):
        in_maps.append({"emb1t": emb1t[b], "emb2t": emb2t[b], **prep})
    res = run_bass_kernel_spmd(nc, in_maps, core_ids=list(range(NCORES)))
    return np.stack([res.results[i]["out"] for i in range(NCORES)],
                    axis=0).astype(np.float32)


# revision 32
# speedup vs baseline: 1.4394x; 1.0252x over previous
"""BiDAF attention-flow kernel for Trainium2 (8 NeuronCores, data-parallel over batch).

Per core (one batch element):
  s[j,i]   = c[j] + q[i] + sum_h w_cq[h]*emb2[j,h]*emb1[i,h]
  a        = softmax_i(s)          (c[j] drops out of the row softmax)
  y2x      = a @ emb1
  b_att    = softmax_j(max_i s)
  x2y      = sum_j b_att[j]*emb2[j]
  out      = [emb2, y2x, emb2*y2x, emb2*x2y] @ w_red + b_red

Implementation notes:
  - b_c/b_q/b_cq cancel exactly in both softmaxes (row/column constants).
  - Row softmax uses a FIXED exp shift (s - SHIFT); true row max recovered as
    SHIFT + ln(max_i u) for b_att.
  - The s-matmul, y2x and pass-1 run on fp8(e4m3) with DoubleRow perf mode
    (2 K-planes per instruction, ~1.8x over bf16 at N=512).  q, c, pass-2 and
    u stay bf16: fp8 there costs ~1-3% output error for little speed.
    Weights are pre-scaled x16 on the host so their fp8 encodings stay in the
    normal range; the 1/16 descale is folded into the exp scale / output STTs.
  - Softmax normalization is folded into the PE transpose of u: transposing
    against diag(1/Z_j) instead of the identity yields normalized a^T free.
  - y2x is computed per GROUP of 4 j-tiles as N=512 DoubleRow sweeps (DR needs
    FD>=256 to beat fast-weight-load bf16).
  - Embeddings arrive TRANSPOSED from the host ([H, L]): DMA rows are then
    4KB (vs 1.5KB), ~2.7x fewer descriptors on the packet-rate-limited DMA
    queues, and e2^T needs no on-device transposing at all.  w1..w4 travel as
    one packed [H, 4*OUTP] tensor for the same reason.  Natural-layout copies
    (e1n for y2x, e2n for x2y) are re-derived by PE transposes, which are
    REGULAR matmuls against an identity: transpose-mode does not count as PE
    activity for the HAM clock gate, real matmuls do, so the load phase warms
    the clock to 2.4 GHz and in-loop e2n transposes keep it there.
  - Main loop is software-pipelined (A = s/exp/stats, T = u transposes, Y =
    grouped y2x, C = pass-1) so the in-order PE queue never head-blocks on a
    fresh dependency; idle >3.4us would re-throttle the clock to 1.2 GHz.
"""

import numpy as np
import ml_dtypes

P = 128
XL = 2048
YL = 2048
H = 768
OUT = 300
OUTP = 320      # OUT padded to a 16B-aligned fp8 stride for DoubleRow
NJT = YL // P   # 16 j tiles
NIC = XL // P   # 16 i chunks
NHC = H // P    # 6 h chunks
SLAB = 512
NSLAB = XL // SLAB  # 4
NCORES = 8
SHIFT = 2.0     # fixed exp shift; keeps u = exp(s-SHIFT) in fp8/bf16 range
WS = 16.0       # host-side weight scale (wq, wc, wcq, w1..w4)
BS = 64.0       # b_att fp8 scale

_CACHE = {}


def _fix_waits(nc, mybir, max_waits=1):
    """This walrus build rejects >1 sync wait per instruction.

    Pass 1: drop waits that are transitively implied by another wait on the
    same instruction.  Pass 2: hoist remaining extra waits onto same-engine
    NoOps inserted right before the instruction.
    """
    from collections import defaultdict

    blocks = [bb for f in nc.m.functions for bb in f.blocks]
    insts = [ins for bb in blocks for ins in bb.instructions]

    dma_types = ("InstDMACopy", "InstDmaTransposeAnt")
    eng_stream = defaultdict(list)
    queue_stream = defaultdict(list)
    sem_events = defaultdict(list)
    cum = defaultdict(int)
    for i, ins in enumerate(insts):
        eng_stream[str(ins.engine)].append(i)
        si = ins.sync_info
        if si and si.on_update:
            for u in si.on_update:
                cum[u.id] += u.update_value
                sem_events[u.id].append((cum[u.id], i))
                if type(ins).__name__ in dma_types:
                    queue_stream[u.id].append(i)

    def achiever(sem_id, val):
        for cv, i in sem_events.get(sem_id, []):
            if cv >= val:
                return i
        return None

    eng_pos, q_pos = {}, {}
    for e, lst in eng_stream.items():
        for k, i in enumerate(lst):
            eng_pos[i] = (e, k)
    for s, lst in queue_stream.items():
        for k, i in enumerate(lst):
            q_pos[i] = (s, k)

    memo = {}

    def implied(i):
        if i in memo:
            return memo[i]
        memo[i] = set()
        out = {i}
        ins = insts[i]
        if i in q_pos:
            s, k = q_pos[i]
            if k > 0:
                out |= implied(queue_stream[s][k - 1])
        e, k = eng_pos[i]
        j = k - 1
        while j >= 0:
            p = eng_stream[e][j]
            if type(insts[p]).__name__ in dma_types:
                j -= 1
                continue
            out |= implied(p)
            break
        si = ins.sync_info
        if si and si.on_wait:
            for w in si.on_wait:
                a = achiever(w.id, w.wait_value)
                if a is not None:
                    out |= implied(a)
        memo[i] = out
        return out

    # pass 1: redundancy elimination
    for i, ins in enumerate(insts):
        si = ins.sync_info
        if not (si and si.on_wait and len(si.on_wait) > max_waits):
            continue
        waits = list(si.on_wait)
        ach = [(w, achiever(w.id, w.wait_value)) for w in waits]
        keep = []
        for wi, (w, a) in enumerate(ach):
            red = False
            if a is not None:
                for wj, (w2, a2) in enumerate(ach):
                    if wi != wj and a2 is not None and a != a2 and a in implied(a2):
                        red = True
                        break
            if not red:
                keep.append(w)
        si.on_wait = keep

    # pass 2: hoist extras onto same-engine NoOps
    k = 0
    for bb in blocks:
        lst = bb.instructions
        i = 0
        while i < len(lst):
            ins = lst[i]
            si = ins.sync_info
            if si and si.on_wait and len(si.on_wait) > max_waits:
                waits = list(si.on_wait)
                extra, keep = waits[:-max_waits], waits[-max_waits:]
                si.on_wait = keep
                nops = []
                for w in extra:
                    nop = mybir.InstNoOp(name=f"I-waitfix-{k}", ins=[], outs=[])
                    k += 1
                    nop.engine = ins.engine
                    nop.sync_info = mybir.SyncInfo(on_wait=[w], on_update=[])
                    nops.append(nop)
                lst[i:i] = nops
                i += len(nops)
            i += 1


def _build():
    import concourse.bass as bass
    import concourse.tile as tile
    import concourse.mybir as mybir
    from concourse.masks import make_identity

    f32 = mybir.dt.float32
    bf16 = mybir.dt.bfloat16
    fp8 = mybir.dt.float8e4
    DR = mybir.MatmulPerfMode.DoubleRow
    MUL = mybir.AluOpType.mult
    ADD = mybir.AluOpType.add
    MAX = mybir.AluOpType.max
    EXP = mybir.ActivationFunctionType.Exp
    LN = mybir.ActivationFunctionType.Ln
    AXX = mybir.AxisListType.X

    nc = bass.Bass("TRN2", target_bir_lowering=False, debug=False,
                   num_devices=NCORES)

    emb1t_d = nc.dram_tensor("emb1t", [H, XL], bf16, kind="ExternalInput")
    emb2t_d = nc.dram_tensor("emb2t", [H, YL], bf16, kind="ExternalInput")
    wc_d = nc.dram_tensor("wc", [P, NHC], bf16, kind="ExternalInput")
    wq_d = nc.dram_tensor("wq", [P, NHC], bf16, kind="ExternalInput")
    wcq_d = nc.dram_tensor("wcq", [P, NHC], f32, kind="ExternalInput")
    wall_d = nc.dram_tensor("wall", [H, 4 * OUTP], bf16, kind="ExternalInput")
    bred_d = nc.dram_tensor("bred", [1, OUTP], f32, kind="ExternalInput")
    out_d = nc.dram_tensor("out", [YL, OUT], bf16, kind="ExternalOutput")

    with tile.TileContext(nc) as tc:
        with (
            tc.tile_pool(name="res", bufs=1) as res,        # resident data
            tc.tile_pool(name="stage", bufs=3) as stage,    # out staging
            tc.tile_pool(name="small", bufs=1) as small,    # stats etc
            tc.tile_pool(name="pss", bufs=2, space="PSUM") as pss,
            tc.tile_pool(name="ptp", bufs=2, space="PSUM") as ptp,
            tc.tile_pool(name="psy", bufs=2, space="PSUM") as psy,
            tc.tile_pool(name="pso", bufs=2, space="PSUM") as pso,
        ):
            # ---- constants ----
            ident16 = res.tile([P, P], bf16, tag="ident16")
            make_identity(nc, ident16)
            ident32 = res.tile([P, P], f32, tag="ident32")
            make_identity(nc, ident32)
            onesBS = res.tile([1, P], f32, tag="onesBS")
            nc.vector.memset(onesBS, BS)
            ones16 = res.tile([1, P], bf16, tag="ones16")
            nc.vector.memset(ones16, 1.0)
            identUS = res.tile([P, P], bf16, tag="identUS")
            nc.vector.tensor_scalar_mul(identUS, ident16, 128.0)
            negC = res.tile([P, 1], f32, tag="negC")
            nc.vector.memset(negC, -SHIFT)
            warm_sb = res.tile([P, OUTP], bf16, tag="warm_sb")
            nc.vector.memset(warm_sb, 0.0)

            # PE warm-up with REAL matmuls: HAM grants 2.4 GHz only after
            # ~3.4us of sustained activity; these bridge t=0 until the load
            # transposes / q matmuls take over.
            _warm_k = [0]

            def warm(n):
                for _ in range(n):
                    wk = _warm_k[0]
                    _warm_k[0] += 1
                    wps = pso.tile([P, OUTP], f32, tag="pso", name=f"warm{wk}")
                    nc.tensor.matmul(wps, ident16, warm_sb, start=True,
                                     stop=True, skip_group_check=True)

            warm(12)

            # ---- small weights ----
            wc_sb = res.tile([P, NHC], bf16, tag="wc")
            nc.sync.dma_start(out=wc_sb, in_=wc_d[:])
            wq_sb = res.tile([P, NHC], bf16, tag="wq")
            nc.sync.dma_start(out=wq_sb, in_=wq_d[:])
            wcq_sb = res.tile([P, NHC], f32, tag="wcq")
            nc.sync.dma_start(out=wcq_sb, in_=wcq_d[:])
            bred_bc = res.tile([P, OUTP], f32, tag="bred_bc")
            _bap = bred_d.ap()
            nc.sync.dma_start(out=bred_bc, in_=bass.AP(
                tensor=_bap.tensor, offset=_bap.offset,
                ap=[[0, P]] + list(_bap.ap[1:])))

            # ---- transposed embeddings: direct DMA, 4KB rows ----
            # e2tt bf16 resident; e2ts = e2tt * (16 w_cq) fp8; e1tt fp8.
            # e1n (fp8 natural pairs, y2x stationary) via PE transposes.
            # e2n (fp8 natural pairs, x2y stationary) transposed in-loop.
            e2tt = res.tile([P, NHC, YL], bf16, tag="e2tt")
            e2ts = res.tile([P, NHC, YL], fp8, tag="e2ts")
            e1tt = res.tile([P, NHC, XL], fp8, tag="e1tt")
            e1n = [res.tile([P, 2, H], fp8, tag=f"e1n{i}", name=f"e1n{i}")
                   for i in range(NIC // 2)]
            e2n = [res.tile([P, 2, H], fp8, tag=f"e2n{i}", name=f"e2n{i}")
                   for i in range(NJT // 2)]
            e1ttbq = res.tile([P, NHC, XL], bf16, tag="e1ttb")
            e1ttb = e1ttbq

            for hc in range(NHC):
                nc.sync.dma_start(out=e2tt[:, hc, :],
                                  in_=emb2t_d[hc * P:(hc + 1) * P, :])
                nc.vector.tensor_scalar_mul(
                    e2ts[:, hc, :], e2tt[:, hc, :], wcq_sb[:, hc:hc + 1])
            for hc in range(NHC):
                nc.sync.dma_start(out=e1ttb[:, hc, :],
                                  in_=emb1t_d[hc * P:(hc + 1) * P, :])

            # q row (bf16): q^T(x16) = (16 w_q)^T @ emb1^T
            qrow = small.tile([1, XL], bf16, tag="qrow")
            for sl in range(NSLAB):
                ssl = slice(sl * SLAB, (sl + 1) * SLAB)
                qp = pss.tile([1, SLAB], f32, tag="pss", name=f"qp{sl}")
                for hc in range(NHC):
                    nc.tensor.matmul(qp, wq_sb[:, hc:hc + 1],
                                     e1ttb[:, hc, ssl],
                                     start=(hc == 0), stop=(hc == NHC - 1),
                                     skip_group_check=True)
                nc.any.tensor_copy(out=qrow[:, ssl], in_=qp)
                warm(1)

            # e1tt fp8 cast on scalar (2/3) + vector (1/3); gpsimd measures
            # ~10x below spec on big tensor ops -- never give it bulk work
            for hc in range(NHC):
                for half in range(2):
                    hsl = slice(half * XL // 2, (half + 1) * XL // 2)
                    if (2 * hc + half) % 3 == 2:
                        nc.vector.tensor_copy(out=e1tt[:, hc, hsl],
                                              in_=e1ttb[:, hc, hsl])
                    else:
                        nc.scalar.copy(out=e1tt[:, hc, hsl],
                                       in_=e1ttb[:, hc, hsl])

            # packed w1..w4 [H, 4*OUTP] bf16; w2/w3 also cast fp8 for DR
            w_all = res.tile([P, NHC, 4 * OUTP], bf16, tag="w_all")
            w2q = res.tile([P, NHC, OUTP], fp8, tag="w2q")
            w3q = res.tile([P, NHC, OUTP], fp8, tag="w3q")
            for hc in range(NHC):
                nc.sync.dma_start(out=w_all[:, hc, :],
                                  in_=wall_d[hc * P:(hc + 1) * P, :])
            for hc in range(NHC):
                nc.any.tensor_copy(out=w2q[:, hc, :],
                                   in_=w_all[:, hc, OUTP:2 * OUTP])
                nc.any.tensor_copy(out=w3q[:, hc, :],
                                   in_=w_all[:, hc, 2 * OUTP:3 * OUTP])

            # ---- stats tiles ----
            M_sb = small.tile([P, NJT], f32, tag="M")
            c_sb = small.tile([P, NJT], f32, tag="c_sb")
            Z_sb = small.tile([P, NJT], f32, tag="Z")
            rZ_sb = small.tile([P, NJT], f32, tag="rZ")
            out_sb = res.tile([P, NJT, OUTP], f32, tag="out_sb")

            # all c columns up-front (PE filler during the load):
            # c^T = e2tt^T @ (16 w_c), then /16
            for jt in range(NJT):
                jsl = slice(jt * P, (jt + 1) * P)
                cp = pss.tile([P, 1], f32, tag="pss", name=f"cp{jt}")
                for hc in range(NHC):
                    nc.tensor.matmul(cp, e2tt[:, hc, jsl], wc_sb[:, hc:hc + 1],
                                     start=(hc == 0), stop=(hc == NHC - 1),
                                     skip_group_check=True)
                nc.any.tensor_scalar_mul(c_sb[:, jt:jt + 1], cp, 1.0 / WS)

            # natural-layout transposes, drip-fed into the loop as PE filler:
            # all of e1n (y2x stationary) must land before Y(0); e2n (x2y
            # stationary) is only needed in the tail.
            trans_q = ([("e1", ic) for ic in range(NIC)] +
                       [("e2", jc) for jc in range(NJT)])

            def drip_trans(n):
                for _ in range(n):
                    if not trans_q:
                        return
                    kind, ck = trans_q.pop(0)
                    src_tt = e1ttbq if kind == "e1" else e2tt
                    dst = e1n if kind == "e1" else e2n
                    for b in range(2):
                        ps = ptp.tile([P, 3, P], f32, tag="ptp",
                                      name=f"nt{kind}{ck}_{b}")
                        for k in range(3):
                            hc = 3 * b + k
                            nc.tensor.matmul(
                                ps[:, k, :],
                                src_tt[:, hc, ck * P:(ck + 1) * P],
                                ident16, start=True, stop=True,
                                skip_group_check=True)
                        nc.any.tensor_copy(
                            out=dst[ck // 2][:, ck % 2,
                                             3 * b * P:(3 * b + 3) * P],
                            in_=ps)

            # ---- main loop: software-pipelined, y2x grouped by 4 tiles ----
            sjt_cm = tc.tile_pool(name="sjt", bufs=2)
            sjt = sjt_cm.__enter__()
            sg4_cm = tc.tile_pool(name="sg4", bufs=2)
            sg4 = sg4_cm.__enter__()
            tiles = {}
            gtiles = {}

            def a_phase(jt):
                jsl = slice(jt * P, (jt + 1) * P)
                # 16*s = 16*q + (e2*16wcq) @ e1^T; u = exp(16s/16 - SHIFT)
                u = sjt.tile([P, XL], bf16, tag="u", name=f"u{jt}")
                Zp = sjt.tile([P, NSLAB], f32, tag="Zp", name=f"Zp{jt}")
                for sl in range(NSLAB):
                    ssl = slice(sl * SLAB, (sl + 1) * SLAB)
                    sp = pss.tile([P, SLAB], f32, tag="pss", name=f"sp{jt}_{sl}")
                    nc.tensor.matmul(sp, ones16, qrow[:, ssl],
                                     start=True, stop=False,
                                     skip_group_check=True)
                    for hp in range(NHC // 2):
                        nc.tensor.matmul(
                            sp, e2ts[:, 2 * hp:2 * hp + 2, jsl],
                            e1tt[:, 2 * hp:2 * hp + 2, ssl],
                            start=False, stop=(hp == NHC // 2 - 1),
                            perf_mode=DR, skip_group_check=True)
                    nc.scalar.activation(out=u[:, ssl], in_=sp, func=EXP,
                                         bias=negC, scale=1.0 / WS,
                                         accum_out=Zp[:, sl:sl + 1])
                drip_trans(4 if jt < 5 else 2)
                nc.vector.tensor_reduce(out=Z_sb[:, jt:jt + 1], in_=Zp,
                                        axis=AXX, op=ADD)
                nc.vector.reciprocal(out=rZ_sb[:, jt:jt + 1],
                                     in_=Z_sb[:, jt:jt + 1])

                # row max for b_att: M = c + SHIFT + ln(max u)
                umax = sjt.tile([P, 1], f32, tag="umax", name=f"umax{jt}")
                nc.vector.tensor_reduce(out=umax, in_=u, axis=AXX, op=MAX)
                lnu = sjt.tile([P, 1], f32, tag="lnu", name=f"lnu{jt}")
                nc.scalar.activation(out=lnu, in_=umax, func=LN)
                nc.vector.scalar_tensor_tensor(
                    out=M_sb[:, jt:jt + 1], in0=lnu, scalar=SHIFT,
                    in1=c_sb[:, jt:jt + 1], op0=ADD, op1=ADD)

                # normalized a^T transpose operand: diag(128/Z_j) -- the
                # x128 keeps small attention weights out of fp8 denormals
                diag = sjt.tile([P, P], bf16, tag="diag", name=f"diag{jt}")
                nc.vector.tensor_scalar_mul(diag, identUS, rZ_sb[:, jt:jt + 1])
                tiles[jt] = (u, diag)

            def get_uT4(g):
                if g not in gtiles:
                    gtiles[g] = sg4.tile([P, NIC, 4, P], fp8, tag="uT4",
                                         name=f"uT4_{g}")
                return gtiles[g]

            def t_phase(jt):
                u, diag = tiles.pop(jt)
                uT4 = get_uT4(jt // 4)
                jj = jt % 4
                # uT4[i, ic, jj, j] = u[j, i] * rZ_j, fp8 cast on the copy
                for g in range(NIC // 4):
                    tp = ptp.tile([P, 4, P], f32, tag="ptp", name=f"tp{jt}_{g}")
                    for k in range(4):
                        ic = g * 4 + k
                        nc.tensor.matmul(tp[:, k, :], u[:, ic * P:(ic + 1) * P],
                                         diag, start=True, stop=True,
                                         skip_group_check=True)
                    nc.any.tensor_copy(out=uT4[:, g * 4:(g + 1) * 4, jj, :],
                                       in_=tp)

            def y_phase(g):
                uT4 = gtiles.pop(g)
                gsl = slice(g * 4 * P, (g + 1) * 4 * P)
                y2xT4 = sg4.tile([P, NHC, 4 * P], fp8, tag="y2xT4",
                                 name=f"y2xT4_{g}")
                bl34 = sg4.tile([P, NHC, 4 * P], fp8, tag="bl34",
                                name=f"bl34_{g}")
                for hc in range(NHC):
                    yp = psy.tile([P, 4 * P], f32, tag="psy", name=f"yp{g}_{hc}")
                    for icp in range(NIC // 2):
                        nc.tensor.matmul(
                            yp,
                            e1n[icp][:, :, hc * P:(hc + 1) * P],
                            uT4[:, 2 * icp:2 * icp + 2, :, :],
                            start=(icp == 0), stop=(icp == NIC // 2 - 1),
                            perf_mode=DR, skip_group_check=True)
                    nc.vector.tensor_scalar_mul(y2xT4[:, hc, :], yp,
                                                 1.0 / 128.0)
                    nc.vector.tensor_mul(bl34[:, hc, :], e2tt[:, hc, gsl],
                                         y2xT4[:, hc, :])
                gtiles[(g, "y")] = (y2xT4, bl34)

            def c_phase(jt):
                g, jj = jt // 4, jt % 4
                y2xT4, bl34 = gtiles[(g, "y")]
                jsl4 = slice(jj * P, (jj + 1) * P)
                # pass-1 reduction: [y2x; e2*y2x] @ 16*[w2; w3] (DoubleRow)
                op1 = pso.tile([P, OUTP], f32, tag="pso", name=f"op1_{jt}")
                for hp in range(NHC // 2):
                    nc.tensor.matmul(op1, y2xT4[:, 2 * hp:2 * hp + 2, jsl4],
                                     w2q[:, 2 * hp:2 * hp + 2, :],
                                     start=(hp == 0), stop=False,
                                     perf_mode=DR, skip_group_check=True)
                for hp in range(NHC // 2):
                    nc.tensor.matmul(op1, bl34[:, 2 * hp:2 * hp + 2, jsl4],
                                     w3q[:, 2 * hp:2 * hp + 2, :],
                                     start=False, stop=(hp == NHC // 2 - 1),
                                     perf_mode=DR, skip_group_check=True)
                # out_sb = psum/16 + b_red
                nc.vector.scalar_tensor_tensor(
                    out=out_sb[:, jt, :], in0=op1, scalar=1.0 / WS,
                    in1=bred_bc, op0=MUL, op1=ADD)
                if jj == 3:
                    gtiles.pop((g, "y"))

            # prologue: A0..A4 interleaved with T0..T3; warm filler covers
            # the load->loop transition so the HAM clock never re-throttles
            a_phase(0)
            warm(3)
            for jt in range(1, 5):
                a_phase(jt)
                warm(2)
                t_phase(jt - 1)
            post_cm = tc.tile_pool(name="post", bufs=1)
            post = post_cm.__enter__()
            for g in range(4):
                y_phase(g)
                if g < 3:
                    for jj in range(4):
                        jt = 4 * (g + 1) + jj
                        if jt + 1 < NJT:
                            a_phase(jt + 1)
                        t_phase(jt)
                    for jj in range(4):
                        c_phase(4 * g + jj)
                else:
                    # epilogue: b_att chain + x2y overlap the last c-phases
                    # ---- b_att = softmax_j(M), no max shift ----
                    bexp = post.tile([P, NJT], f32, tag="bexp")
                    brow = post.tile([P, 1], f32, tag="brow")
                    nc.scalar.activation(out=bexp, in_=M_sb, func=EXP,
                                         accum_out=brow)
                    tpb = pss.tile([1, P], f32, tag="pss", name="tpb")
                    nc.tensor.transpose(tpb, brow, ident32)
                    brw = post.tile([1, P], f32, tag="brw")
                    nc.vector.tensor_copy(out=brw, in_=tpb)
                    bs0 = post.tile([1, 1], f32, tag="bs0")
                    nc.vector.tensor_reduce(out=bs0, in_=brw, axis=AXX, op=ADD)
                    rb0 = post.tile([1, 1], f32, tag="rb0")
                    nc.vector.reciprocal(rb0, bs0)
                    rbp = pss.tile([P, 1], f32, tag="pss", name="rbp")
                    nc.tensor.matmul(rbp, onesBS, rb0, start=True, stop=True,
                                     skip_group_check=True)
                    rbz = post.tile([P, 1], f32, tag="rbz")
                    nc.vector.tensor_copy(out=rbz, in_=rbp)
                    battq = post.tile([P, NJT], fp8, tag="battq")
                    nc.vector.tensor_scalar_mul(battq, bexp, rbz)

                    c_phase(12)
                    c_phase(13)

                    # x2y^T: x2yT[h] = sum_j e2n[j,h] * (64 b_j), then /64
                    x2p = psy.tile([P, NHC], f32, tag="psy", name="x2p")
                    for hc in range(NHC):
                        for jc in range(NJT):
                            nc.tensor.matmul(
                                x2p[:, hc:hc + 1],
                                e2n[jc // 2][:, jc % 2, hc * P:(hc + 1) * P],
                                battq[:, jc:jc + 1],
                                start=(jc == 0), stop=(jc == NJT - 1),
                                skip_group_check=True)
                    c_phase(14)
                    c_phase(15)
                    x2yT = post.tile([P, NHC], f32, tag="x2yT")
                    nc.vector.tensor_scalar_mul(x2yT, x2p, 1.0 / BS)

                    # w14 = 16*(w1 + x2y*w4), bf16
                    w14 = res.tile([P, NHC, OUTP], bf16, tag="w14")
                    for hc in range(NHC):
                        nc.vector.scalar_tensor_tensor(
                            out=w14[:, hc, :],
                            in0=w_all[:, hc, 3 * OUTP:4 * OUTP],
                            scalar=x2yT[:, hc:hc + 1],
                            in1=w_all[:, hc, 0:OUTP],
                            op0=MUL, op1=ADD)
                    warm(6)

            post_cm.__exit__(None, None, None)
            sg4_cm.__exit__(None, None, None)
            sjt_cm.__exit__(None, None, None)

            # ---- pass 2: out += emb2 @ w14/16 (bf16), stream out ----
            for jt in range(NJT):
                jsl = slice(jt * P, (jt + 1) * P)
                op2 = pso.tile([P, OUTP], f32, tag="pso", name=f"op2_{jt}")
                for hc in range(NHC):
                    nc.tensor.matmul(op2, e2tt[:, hc, jsl], w14[:, hc, :],
                                     start=(hc == 0), stop=(hc == NHC - 1),
                                     skip_group_check=True)
                fin = stage.tile([P, OUTP], bf16, tag="fin", name=f"fin{jt}")
                nc.vector.scalar_tensor_tensor(
                    out=fin, in0=op2, scalar=1.0 / WS,
                    in1=out_sb[:, jt, :], op0=MUL, op1=ADD)
                nc.sync.dma_start(out=out_d[jsl, :], in_=fin[:, 0:OUT])

    return nc


def _get_nc(drain_fix=True):
    if "nc" not in _CACHE:
        _CACHE["nc"] = _build()
    if drain_fix and not _CACHE.get("drain_fixed"):
        import concourse.mybir as mybir
        _fix_waits(_CACHE["nc"], mybir, max_waits=1)
        _CACHE["drain_fixed"] = True
    return _CACHE["nc"]


def _prep_weights(w_c, w_q, w_cq, w_red, b_red):
    bf = ml_dtypes.bfloat16
    w_red = np.asarray(w_red, dtype=np.float32)

    wall = np.zeros((H, 4 * OUTP), np.float32)
    for k in range(4):
        wall[:, k * OUTP:k * OUTP + OUT] = w_red[k * H:(k + 1) * H]
    bredp = np.zeros((1, OUTP), np.float32)
    bredp[0, :OUT] = np.asarray(b_red, np.float32)
    return {
        "wc": np.ascontiguousarray(
            (np.asarray(w_c, np.float32) * WS).reshape(NHC, P).T.astype(bf)),
        "wq": np.ascontiguousarray(
            (np.asarray(w_q, np.float32) * WS).reshape(NHC, P).T.astype(bf)),
        "wcq": np.ascontiguousarray(
            (np.asarray(w_cq, np.float32) * WS).reshape(NHC, P).T),
        "wall": np.ascontiguousarray((wall * WS).astype(bf)),
        "bred": np.ascontiguousarray(bredp),
    }


def kernel(emb1, emb2, w_c, b_c, w_q, b_q, w_cq, b_cq, w_red, b_red):
    from concourse.bass_utils import run_bass_kernel_spmd

    nc = _get_nc()
    bf = ml_dtypes.bfloat16

    # host-side transpose: DMA rows become 4KB (packet-rate-limited queues)
    emb1t = np.ascontiguousarray(
        np.asarray(emb1, dtype=np.float32).transpose(0, 2, 1).astype(bf))
    emb2t = np.ascontiguousarray(
        np.asarray(emb2, dtype=np.float32).transpose(0, 2, 1).astype(bf))

    # b_c, b_q, b_cq cancel exactly in both softmaxes (per-row/col consts).
    prep = _prep_weights(w_c, w_q, w_cq, w_red, b_red)

    in_maps = []
    for b in range(NCORES):
        in_maps.append({"emb1t": emb1t[b], "emb2t": emb2t[b], **prep})
    res = run_bass_kernel_spmd(nc, in_maps, core_ids=list(range(NCORES)))
    return np.stack([res.results[i]["out"] for i in range(NCORES)],
                    axis=0).astype(np.float32)
